# revision 24
# baseline (speedup 1.0000x reference)
"""Trainium2 Bass kernel for nn_MultiHeadAttention_91027536871977.

Cosine-similarity multi-head self-attention:
  x      = einsum("bsd,hdf->bhsf", sin, Wx) + bx          [B,H,S,F]
  scores = (x @ x^T) / (|x| |x|^T)                        [B,H,S,S]
  p      = softmax(scores, -1)
  out    = concat_heads(p @ x) @ Wp + bp                  [B,S,D]

Sharding: pure data-parallel over batch (B=8 -> 8 cores, one batch each,
all 16 heads + the output projection local to the core; no collectives).

Per-core algorithm (S=1024, D=1024, H=16, F=64, P=128):
  - host pre-transposes sin[b] -> sinT [D,S] and casts weights to bf16
  - XT[f2,s] per head-pair q (2 heads stacked on 128 partitions) via matmul
  - n^2 per head via (XT*XT) summed over the 64 feature partitions with a
    0/1-mask matmul; inv_n = sqrt(1/n^2); XTn = XT * inv_n (cols scaled)
  - Gram G = XTn^T XTn per head (K=64, two heads run concurrently on
    disjoint PE row-groups) gives fully normalized scores (symmetric)
  - E = exp(G) on ScalarE, PSUM->SBUF bf16, with accum_out giving row sums
    rs for free.  (The reference's `score==0 -> -inf` quirk fires on ~4 of
    134M elements in fp32 and is numerically negligible; ignored.)
  - out^T = X^T E / rs using E's symmetry (stored [s,t] tiles reinterpreted
    as [t,s]), X = sin @ Wx computed directly in [t, hf] layout.
    1/rs is laid out via a PE transpose of the accumulated rs matrix and
    broadcast across partitions with K=1 ones-matmuls.
  - Y = out^T.T @ Wp + bp via matmul, bias added during PSUM->SBUF copy.
"""

import numpy as np
import ml_dtypes

import concourse.bass as bass
import concourse.bacc as bacc
import concourse.mybir as mybir
import concourse.tile as tile
from concourse.bass_utils import run_bass_kernel_spmd

B, S, D, H, F = 8, 1024, 1024, 16, 64
P = 128
NP = H // 2  # head pairs
KO = D // P  # k subtiles
NT = S // P  # s tiles
BF16 = mybir.dt.bfloat16
F32 = mybir.dt.float32


def build_program() -> bass.Bass:
    nc = bacc.Bacc("TRN2", target_bir_lowering=False, debug=False)

    # Per-core inputs (already sharded/prepped on host).
    d_sint = nc.dram_tensor("sint", [D, S], BF16, kind="ExternalInput")
    d_wx = nc.dram_tensor("wx", [D, H * F], BF16, kind="ExternalInput")
    d_wp = nc.dram_tensor("wp", [H * F, D], BF16, kind="ExternalInput")
    d_bxp = nc.dram_tensor("bxp", [P, NP], F32, kind="ExternalInput")  # pair bias
    d_bxf = nc.dram_tensor("bxf", [1, H * F], F32, kind="ExternalInput")
    d_bp = nc.dram_tensor("bp", [1, D], F32, kind="ExternalInput")
    d_selp = nc.dram_tensor("selp", [2, P], F32, kind="ExternalInput")
    d_m64 = nc.dram_tensor("m64", [P, 2], F32, kind="ExternalInput")
    d_sel8 = nc.dram_tensor("sel8", [2 * NT, NT, P], F32, kind="ExternalInput")
    d_ident = nc.dram_tensor("ident", [P, P], F32, kind="ExternalInput")
    d_y = nc.dram_tensor("y", [S, D], F32, kind="ExternalOutput")

    with tile.TileContext(nc) as tc:
        _body(tc, d_sint, d_wx, d_wp, d_bxp, d_bxf, d_bp, d_selp, d_m64,
              d_sel8, d_ident, d_y)
    nc.compile()
    return nc


def _bcast_rows(dram_ap, parts=P):
    """DMA access pattern replicating a [1, N] DRAM row across `parts` partitions."""
    return bass.AP(
        tensor=dram_ap.tensor,
        offset=dram_ap.offset,
        ap=[[0, parts]] + list(dram_ap.ap[1:]),
    )


def _body(tc, d_sint, d_wx, d_wp, d_bxp, d_bxf, d_bp, d_selp, d_m64,
          d_sel8, d_ident, d_y):
    nc = tc.nc
    from contextlib import ExitStack

    with ExitStack() as ctx:
        singles = ctx.enter_context(tc.tile_pool(name="singles", bufs=1))
        sq_pool = ctx.enter_context(tc.tile_pool(name="sq", bufs=2))
        e_pool = ctx.enter_context(tc.tile_pool(name="epool", bufs=2))
        b_pool = ctx.enter_context(tc.tile_pool(name="bpool", bufs=2))
        y_pool = ctx.enter_context(tc.tile_pool(name="ypool", bufs=2))

        ps_big = ctx.enter_context(tc.tile_pool(name="ps_big", bufs=2, space="PSUM"))
        ps_out = ctx.enter_context(tc.tile_pool(name="ps_out", bufs=1, space="PSUM"))
        ps_small = ctx.enter_context(tc.tile_pool(name="ps_small", bufs=2, space="PSUM"))

        # ---- load everything to SBUF ----
        sint_sb = singles.tile([P, KO, S], BF16)
        nc.sync.dma_start(sint_sb, d_sint.rearrange("(ko p) s -> p ko s", p=P))
        wx_sb = singles.tile([P, KO, H * F], BF16)
        nc.sync.dma_start(wx_sb, d_wx.rearrange("(ko p) n -> p ko n", p=P))
        wp_sb = singles.tile([P, KO, D], BF16)
        nc.sync.dma_start(wp_sb, d_wp.rearrange("(ko p) n -> p ko n", p=P))
        bxp_sb = singles.tile([P, NP], F32)
        nc.sync.dma_start(bxp_sb, d_bxp[:, :])
        bxf_sb = singles.tile([P, H * F], F32)
        nc.gpsimd.dma_start(bxf_sb, _bcast_rows(d_bxf[:, :]))
        bp_sb = singles.tile([P, D], F32)
        nc.gpsimd.dma_start(bp_sb, _bcast_rows(d_bp[:, :]))
        selp_sb = singles.tile([2, P], F32)
        nc.sync.dma_start(selp_sb, d_selp[:, :])
        m64_sb = singles.tile([P, 2], F32)
        nc.sync.dma_start(m64_sb, d_m64[:, :])
        sel8_sb = singles.tile([2 * NT, NT, P], F32)
        nc.sync.dma_start(sel8_sb, d_sel8[:, :, :])
        ident_sb = singles.tile([P, P], F32)
        nc.sync.dma_start(ident_sb, d_ident[:, :])

        # persistent intermediates
        xt_sb = singles.tile([P, NP, S], BF16)     # [f2, pair, t]  x^T per pair
        xtn_sb = singles.tile([P, NP, S], BF16)    # normalized x^T
        x_sb = singles.tile([P, NT, H * F], BF16)  # [t_p, t_tile, hf]  x values
        outt_sb = singles.tile([P, NP, S], BF16)   # attention out^T [f2, pair, s]
        rs_sb = singles.tile([P, P], F32)          # rs[s_p, h*8+i]
        invn_sb = singles.tile([2, NP, S], F32)    # 1/|x| per pair

        HALF = S // 2

        # ---- head projections: XT per pair, X in [t, hf] layout ----
        for q in range(NP):
            xt_ps = ps_big.tile([P, S], F32, tag="big")
            for hlf in range(2):
                for ko in range(KO):
                    nc.tensor.matmul(
                        xt_ps[:, hlf * HALF:(hlf + 1) * HALF],
                        lhsT=wx_sb[:, ko, q * P:(q + 1) * P],
                        rhs=sint_sb[:, ko, hlf * HALF:(hlf + 1) * HALF],
                        start=(ko == 0), stop=(ko == KO - 1),
                    )
            # += bias (per-partition scalar broadcast along free), cast to bf16
            nc.vector.tensor_add(
                xt_sb[:, q, :], xt_ps, bxp_sb[:, q:q + 1].to_broadcast([P, S]))

            # squares -> n^2 per head via mask matmul
            xtsq = sq_pool.tile([P, S], F32, tag="xtsq")
            nc.vector.tensor_mul(xtsq, xt_sb[:, q, :], xt_sb[:, q, :])
            for hlf in range(2):
                nsq_ps = ps_small.tile([2, HALF], F32, tag="small")
                nc.tensor.matmul(
                    nsq_ps,
                    lhsT=m64_sb,
                    rhs=xtsq[:, hlf * HALF:(hlf + 1) * HALF],
                    start=True, stop=True,
                )
                # inv_n = sqrt(1/n^2)
                nc.vector.reciprocal(
                    invn_sb[:, q, hlf * HALF:(hlf + 1) * HALF], nsq_ps)
            nc.scalar.sqrt(invn_sb[:, q, :], invn_sb[:, q, :])
            # broadcast inv_n across the pair's 128 partitions (K=2 matmul)
            invb_ps = ps_big.tile([P, S], F32, tag="big")
            for hlf in range(2):
                nc.tensor.matmul(
                    invb_ps[:, hlf * HALF:(hlf + 1) * HALF],
                    lhsT=selp_sb,
                    rhs=invn_sb[:, q, hlf * HALF:(hlf + 1) * HALF],
                    start=True, stop=True,
                )
            nc.vector.tensor_mul(xtn_sb[:, q, :], xt_sb[:, q, :], invb_ps)

        for i in range(NT):
            x_ps = ps_big.tile([P, H * F], F32, tag="big")
            for hlf in range(2):
                for ko in range(KO):
                    nc.tensor.matmul(
                        x_ps[:, hlf * HALF:(hlf + 1) * HALF],
                        lhsT=sint_sb[:, ko, i * P:(i + 1) * P],
                        rhs=wx_sb[:, ko, hlf * HALF:(hlf + 1) * HALF],
                        start=(ko == 0), stop=(ko == KO - 1),
                    )
            nc.vector.tensor_add(x_sb[:, i, :], x_ps, bxf_sb[:, :])

        # ---- attention per head; EX + rescale per pair ----
        e_tiles = {}
        for h in range(H):
            q, hh = h // 2, h % 2
            frows = slice(hh * F, (hh + 1) * F)
            e_sb = e_pool.tile([P, NT, S], BF16, tag="e")
            e_tiles[hh] = e_sb
            for i in range(NT):
                g_ps = ps_big.tile([P, S], F32, tag="big")
                for hlf in range(2):
                    nc.tensor.matmul(
                        g_ps[:, hlf * HALF:(hlf + 1) * HALF],
                        lhsT=xtn_sb[frows, q, i * P:(i + 1) * P],
                        rhs=xtn_sb[frows, q, hlf * HALF:(hlf + 1) * HALF],
                        start=True, stop=True,
                    )
                nc.scalar.activation(
                    e_sb[:, i, :], g_ps,
                    mybir.ActivationFunctionType.Exp,
                    accum_out=rs_sb[:, h * NT + i:h * NT + i + 1],
                )

            if hh == 1:
                # transpose this pair's 16 rs columns -> [16, 128] rows
                rst_ps = ps_small.tile([2 * NT, P], F32, tag="small")
                nc.tensor.transpose(
                    rst_ps, rs_sb[:, q * 2 * NT:(q + 1) * 2 * NT], ident_sb)
                rcpq_sb = b_pool.tile([2 * NT, P], F32, tag="rcpq")
                nc.vector.reciprocal(rcpq_sb, rst_ps)

                # out^T accumulation over t tiles, both heads col-packed
                ot_ps = ps_out.tile([P, S], F32, tag="ot")
                for hlf in range(2):
                    for j in range(NT):
                        for hh2 in range(2):
                            nc.tensor.matmul(
                                ot_ps[hh2 * F:(hh2 + 1) * F,
                                      hlf * HALF:(hlf + 1) * HALF],
                                lhsT=x_sb[:, j, (2 * q + hh2) * F:(2 * q + hh2 + 1) * F],
                                rhs=e_tiles[hh2][:, j, hlf * HALF:(hlf + 1) * HALF],
                                start=(j == 0), stop=(j == NT - 1),
                                tile_position=(0, hh2 * F),
                                skip_group_check=True,
                            )
                # broadcast 1/rs across partitions and scale while copying out
                for i in range(NT):
                    brc_ps = ps_small.tile([P, P], F32, tag="small")
                    nc.tensor.matmul(
                        brc_ps,
                        lhsT=sel8_sb[:, i, :],
                        rhs=rcpq_sb,
                        start=True, stop=True,
                    )
                    brc_sb = b_pool.tile([P, P], F32, tag="brc")
                    nc.vector.tensor_copy(brc_sb, brc_ps)
                    nc.vector.tensor_mul(
                        outt_sb[:, q, i * P:(i + 1) * P],
                        brc_sb,
                        ot_ps[:, i * P:(i + 1) * P],
                    )

        # ---- output projection Y = out^T.T @ Wp + bp ----
        for i in range(NT):
            y_ps = ps_big.tile([P, D], F32, tag="big")
            for hlf in range(2):
                for q in range(NP):
                    nc.tensor.matmul(
                        y_ps[:, hlf * HALF:(hlf + 1) * HALF],
                        lhsT=outt_sb[:, q, i * P:(i + 1) * P],
                        rhs=wp_sb[:, q, hlf * HALF:(hlf + 1) * HALF],
                        start=(q == 0), stop=(q == NP - 1),
                    )
            y_sb = y_pool.tile([P, D], F32, tag="y")
            nc.vector.tensor_add(y_sb, y_ps, bp_sb)
            nc.sync.dma_start(d_y[i * P:(i + 1) * P, :], y_sb)


_CACHE: dict = {}


def _get_program() -> bass.Bass:
    if "nc" not in _CACHE:
        _CACHE["nc"] = build_program()
    return _CACHE["nc"]


def _prep_inputs(sin, Wx, bx, Wp, bp):
    """Host-side sharding + layout prep. Returns per-core input maps."""
    bf16 = ml_dtypes.bfloat16
    wx_flat = np.ascontiguousarray(
        np.transpose(np.asarray(Wx, np.float32), (1, 0, 2)).reshape(D, H * F)
    ).astype(bf16)
    wp_b = np.ascontiguousarray(np.asarray(Wp, np.float32)).astype(bf16)
    bx32 = np.asarray(bx, np.float32)
    # bxp[p, q] = bx[2q + p//64, p%64]
    bxp = np.ascontiguousarray(bx32.reshape(NP, P).T)
    bxf = np.ascontiguousarray(bx32.reshape(1, H * F))
    bp32 = np.ascontiguousarray(np.asarray(bp, np.float32).reshape(1, D))
    selp = np.zeros((2, P), np.float32)
    selp[0, :F] = 1.0
    selp[1, F:] = 1.0
    m64 = np.ascontiguousarray(selp.T)
    # sel8[i][k][p] = 1 iff k == (p//64)*8 + i  (broadcasts rcpq rows i and
    # 8+i of a pair's [16,128] 1/rs tile to partitions 0-63 / 64-127)
    sel8 = np.zeros((2 * NT, NT, P), np.float32)
    for i in range(NT):
        sel8[i, i, :F] = 1.0
        sel8[NT + i, i, F:] = 1.0
    ident = np.eye(P, dtype=np.float32)

    sin32 = np.asarray(sin, np.float32)
    in_maps = []
    for b in range(B):
        sint = np.ascontiguousarray(sin32[b].T).astype(bf16)
        in_maps.append({
            "sint": sint, "wx": wx_flat, "wp": wp_b, "bxp": bxp, "bxf": bxf,
            "bp": bp32, "selp": selp, "m64": m64, "sel8": sel8,
            "ident": ident,
        })
    return in_maps


def kernel(sin, mask, Wx, bx, Wp, bp, _run_kwargs=None):
    nc = _get_program()
    in_maps = _prep_inputs(sin, Wx, bx, Wp, bp)
    res = run_bass_kernel_spmd(nc, in_maps, core_ids=list(range(B)),
                               **(_run_kwargs or {}))
    out = np.stack([np.asarray(res.results[b]["y"], np.float32) for b in range(B)])
    if _run_kwargs:
        _CACHE["last_results"] = res
    return out


# revision 28
# speedup vs baseline: 1.0749x; 1.0749x over previous
"""Trainium2 Bass kernel for nn_MultiHeadAttention_91027536871977.

Cosine-similarity multi-head self-attention:
  x      = einsum("bsd,hdf->bhsf", sin, Wx) + bx          [B,H,S,F]
  scores = (x @ x^T) / (|x| |x|^T)                        [B,H,S,S]
  p      = softmax(scores, -1)
  out    = concat_heads(p @ x) @ Wp + bp                  [B,S,D]

Sharding: pure data-parallel over batch (B=8 -> 8 cores, one batch each,
all 16 heads + the output projection local to the core; no collectives).

Per-core algorithm (S=1024, D=1024, H=16, F=64, P=128):
  - host pre-transposes sin[b] -> sinT [D,S] and casts weights to bf16
  - XT[f2,s] per head-pair q (2 heads stacked on 128 partitions) via matmul
  - n^2 per head via (XT*XT) summed over the 64 feature partitions with a
    0/1-mask matmul; inv_n = sqrt(1/n^2); XTn = XT * inv_n (cols scaled)
  - Gram G = XTn^T XTn per head (K=64, two heads run concurrently on
    disjoint PE row-groups) gives fully normalized scores (symmetric)
  - E = exp(G) on ScalarE, PSUM->SBUF bf16, with accum_out giving row sums
    rs for free.  (The reference's `score==0 -> -inf` quirk fires on ~4 of
    134M elements in fp32 and is numerically negligible; ignored.)
  - out^T = X^T E / rs using E's symmetry (stored [s,t] tiles reinterpreted
    as [t,s]), X = sin @ Wx computed directly in [t, hf] layout.
    1/rs is laid out via a PE transpose of the accumulated rs matrix and
    broadcast across partitions with K=1 ones-matmuls.
  - Y = out^T.T @ Wp + bp via matmul, bias added during PSUM->SBUF copy.
"""

import numpy as np
import ml_dtypes

import concourse.bass as bass
import concourse.bacc as bacc
import concourse.mybir as mybir
import concourse.tile as tile
from concourse.bass_utils import run_bass_kernel_spmd

B, S, D, H, F = 8, 1024, 1024, 16, 64
P = 128
NP = H // 2  # head pairs
KO = D // P  # k subtiles
NT = S // P  # s tiles
BF16 = mybir.dt.bfloat16
F32 = mybir.dt.float32


def build_program() -> bass.Bass:
    nc = bacc.Bacc("TRN2", target_bir_lowering=False, debug=False)

    # Per-core inputs (already sharded/prepped on host).
    d_sint = nc.dram_tensor("sint", [D, S], BF16, kind="ExternalInput")
    d_wx = nc.dram_tensor("wx", [D, H * F], BF16, kind="ExternalInput")
    d_wp = nc.dram_tensor("wp", [H * F, D], BF16, kind="ExternalInput")
    d_bxp = nc.dram_tensor("bxp", [P, NP], F32, kind="ExternalInput")  # pair bias
    d_bxf = nc.dram_tensor("bxf", [1, H * F], F32, kind="ExternalInput")
    d_bp = nc.dram_tensor("bp", [1, D], F32, kind="ExternalInput")
    d_sel8 = nc.dram_tensor("sel8", [2 * NT, NT, P], F32, kind="ExternalInput")
    d_ident = nc.dram_tensor("ident", [P, P], F32, kind="ExternalInput")
    d_y = nc.dram_tensor("y", [S, D], F32, kind="ExternalOutput")

    with tile.TileContext(nc) as tc:
        _body(tc, d_sint, d_wx, d_wp, d_bxp, d_bxf, d_bp,
              d_sel8, d_ident, d_y)
    nc.compile()
    return nc


def _bcast_rows(dram_ap, parts=P):
    """DMA access pattern replicating a [1, N] DRAM row across `parts` partitions."""
    return bass.AP(
        tensor=dram_ap.tensor,
        offset=dram_ap.offset,
        ap=[[0, parts]] + list(dram_ap.ap[1:]),
    )


def _body(tc, d_sint, d_wx, d_wp, d_bxp, d_bxf, d_bp,
          d_sel8, d_ident, d_y):
    nc = tc.nc
    from contextlib import ExitStack

    with ExitStack() as ctx:
        singles = ctx.enter_context(tc.tile_pool(name="singles", bufs=1))
        sq_pool = ctx.enter_context(tc.tile_pool(name="sq", bufs=2))
        e_pool = ctx.enter_context(tc.tile_pool(name="epool", bufs=2))
        b_pool = ctx.enter_context(tc.tile_pool(name="bpool", bufs=2))
        y_pool = ctx.enter_context(tc.tile_pool(name="ypool", bufs=2))

        ps_big = ctx.enter_context(tc.tile_pool(name="ps_big", bufs=2, space="PSUM"))
        ps_out = ctx.enter_context(tc.tile_pool(name="ps_out", bufs=1, space="PSUM"))
        ps_small = ctx.enter_context(tc.tile_pool(name="ps_small", bufs=2, space="PSUM"))

        # ---- load everything to SBUF ----
        sint_sb = singles.tile([P, KO, S], BF16)
        nc.sync.dma_start(sint_sb, d_sint.rearrange("(ko p) s -> p ko s", p=P))
        wx_sb = singles.tile([P, KO, H * F], BF16)
        nc.sync.dma_start(wx_sb, d_wx.rearrange("(ko p) n -> p ko n", p=P))
        wp_sb = singles.tile([P, KO, D], BF16)
        nc.sync.dma_start(wp_sb, d_wp.rearrange("(ko p) n -> p ko n", p=P))
        bxp_sb = singles.tile([P, NP], F32)
        nc.sync.dma_start(bxp_sb, d_bxp[:, :])
        bxf_sb = singles.tile([P, H * F], F32)
        nc.gpsimd.dma_start(bxf_sb, _bcast_rows(d_bxf[:, :]))
        bp_sb = singles.tile([P, D], F32)
        nc.gpsimd.dma_start(bp_sb, _bcast_rows(d_bp[:, :]))
        sel8_sb = singles.tile([2 * NT, NT, P], F32)
        nc.sync.dma_start(sel8_sb, d_sel8[:, :, :])
        ident_sb = singles.tile([P, P], F32)
        nc.sync.dma_start(ident_sb, d_ident[:, :])

        # persistent intermediates
        xt_sb = singles.tile([P, NP, S], BF16)     # [f2, pair, t]  x^T per pair
        xtn_sb = singles.tile([P, NP, S], BF16)    # normalized x^T
        x_sb = singles.tile([P, NT, H * F], BF16)  # [t_p, t_tile, hf]  x values
        outt_sb = singles.tile([P, NP, S], BF16)   # attention out^T [f2, pair, s]
        rs_sb = singles.tile([P, P], F32)          # rs[s_p, col h*8+i]
        n2s_sb = singles.tile([P, P], F32)         # |x|^2 [s_p, col h*8+i]
        invs_sb = singles.tile([P, P], F32)        # 1/|x| [s_p, col h*8+i]

        HALF = S // 2

        # ---- X = sin @ Wx + bx in [t, hf] layout, + per-head |x|^2 ----
        for i in range(NT):
            x_ps = ps_big.tile([P, H * F], F32, tag="big")
            for hlf in range(2):
                for ko in range(KO):
                    nc.tensor.matmul(
                        x_ps[:, hlf * HALF:(hlf + 1) * HALF],
                        lhsT=sint_sb[:, ko, i * P:(i + 1) * P],
                        rhs=wx_sb[:, ko, hlf * HALF:(hlf + 1) * HALF],
                        start=(ko == 0), stop=(ko == KO - 1),
                    )
            nc.vector.tensor_add(x_sb[:, i, :], x_ps, bxf_sb[:, :])
            xsq = sq_pool.tile([P, H * F], F32, tag="xsq")
            nc.vector.tensor_mul(xsq, x_sb[:, i, :], x_sb[:, i, :])
            # reduce over F per head -> [s_p, 16]; scatter to cols h*8+i
            nc.vector.reduce_sum(
                n2s_sb.rearrange("p (hh ii) -> p hh ii", ii=NT)[:, :, i],
                xsq.rearrange("p (hh f) -> p hh f", f=F),
                axis=mybir.AxisListType.X,
            )
        # 1/|x| for all heads/tiles at once (full-partition ops are fast)
        nc.vector.reciprocal(invs_sb, n2s_sb)
        nc.scalar.sqrt(invs_sb, invs_sb)

        # ---- per pair: XT, normalized XT, attention, out^T ----
        e_tiles = {}
        for q in range(NP):
            # XT via matmul (f2 on partitions)
            xt_ps = ps_big.tile([P, S], F32, tag="big")
            for hlf in range(2):
                for ko in range(KO):
                    nc.tensor.matmul(
                        xt_ps[:, hlf * HALF:(hlf + 1) * HALF],
                        lhsT=wx_sb[:, ko, q * P:(q + 1) * P],
                        rhs=sint_sb[:, ko, hlf * HALF:(hlf + 1) * HALF],
                        start=(ko == 0), stop=(ko == KO - 1),
                    )
            nc.vector.tensor_add(
                xt_sb[:, q, :], xt_ps, bxp_sb[:, q:q + 1].to_broadcast([P, S]))

            # 1/|x| free-layout rows for this pair: transpose [128,16] slice
            invq_ps = ps_small.tile([2 * NT, P], F32, tag="small")
            nc.tensor.transpose(
                invq_ps, invs_sb[:, q * 2 * NT:(q + 1) * 2 * NT], ident_sb)
            invq_sb = b_pool.tile([2 * NT, P], F32, tag="rcpq")
            nc.vector.tensor_copy(invq_sb, invq_ps)
            # broadcast across the pair's partitions, tile by tile, and scale
            nrm_sb = sq_pool.tile([P, NT, P], F32, tag="nrm")
            for i in range(NT):
                nrm_ps = ps_small.tile([P, P], F32, tag="small")
                nc.tensor.matmul(
                    nrm_ps, lhsT=sel8_sb[:, i, :], rhs=invq_sb,
                    start=True, stop=True,
                )
                nc.vector.tensor_copy(nrm_sb[:, i, :], nrm_ps)
            nc.vector.tensor_mul(
                xtn_sb[:, q, :], xt_sb[:, q, :],
                nrm_sb.rearrange("p a b -> p (a b)"))

            # Gram + exp per head (row-packed K=64 matmuls run concurrently)
            for hh in range(2):
                h = 2 * q + hh
                frows = slice(hh * F, (hh + 1) * F)
                e_sb = e_pool.tile([P, NT, S], BF16, tag="e")
                e_tiles[hh] = e_sb
                for i in range(NT):
                    g_ps = ps_big.tile([P, S], F32, tag="big")
                    for hlf in range(2):
                        nc.tensor.matmul(
                            g_ps[:, hlf * HALF:(hlf + 1) * HALF],
                            lhsT=xtn_sb[frows, q, i * P:(i + 1) * P],
                            rhs=xtn_sb[frows, q, hlf * HALF:(hlf + 1) * HALF],
                            start=True, stop=True,
                        )
                    nc.scalar.activation(
                        e_sb[:, i, :], g_ps,
                        mybir.ActivationFunctionType.Exp,
                        accum_out=rs_sb[:, h * NT + i:h * NT + i + 1],
                    )

            # 1/rs rows for the pair (reciprocal on full partitions, then T)
            rcps_sb = b_pool.tile([P, 2 * NT], F32, tag="rcps")
            nc.vector.reciprocal(
                rcps_sb, rs_sb[:, q * 2 * NT:(q + 1) * 2 * NT])
            rst_ps = ps_small.tile([2 * NT, P], F32, tag="small")
            nc.tensor.transpose(rst_ps, rcps_sb, ident_sb)
            rcpq_sb = b_pool.tile([2 * NT, P], F32, tag="rcpq")
            nc.vector.tensor_copy(rcpq_sb, rst_ps)
            # broadcast tiles staged into one [128, S] buffer (single mul below)
            brc_sb = sq_pool.tile([P, NT, P], F32, tag="nrm")
            for i in range(NT):
                brc_ps = ps_small.tile([P, P], F32, tag="small")
                nc.tensor.matmul(
                    brc_ps, lhsT=sel8_sb[:, i, :], rhs=rcpq_sb,
                    start=True, stop=True,
                )
                nc.vector.tensor_copy(brc_sb[:, i, :], brc_ps)

            # out^T accumulation over t tiles, both heads col-packed
            ot_ps = ps_out.tile([P, S], F32, tag="ot")
            for hlf in range(2):
                for j in range(NT):
                    for hh2 in range(2):
                        nc.tensor.matmul(
                            ot_ps[hh2 * F:(hh2 + 1) * F,
                                  hlf * HALF:(hlf + 1) * HALF],
                            lhsT=x_sb[:, j, (2 * q + hh2) * F:(2 * q + hh2 + 1) * F],
                            rhs=e_tiles[hh2][:, j, hlf * HALF:(hlf + 1) * HALF],
                            start=(j == 0), stop=(j == NT - 1),
                            tile_position=(0, hh2 * F),
                            skip_group_check=True,
                        )
            nc.vector.tensor_mul(
                outt_sb[:, q, :],
                brc_sb.rearrange("p a b -> p (a b)"),
                ot_ps,
            )

        # ---- output projection Y = out^T.T @ Wp + bp ----
        for i in range(NT):
            y_ps = ps_big.tile([P, D], F32, tag="big")
            for hlf in range(2):
                for q in range(NP):
                    nc.tensor.matmul(
                        y_ps[:, hlf * HALF:(hlf + 1) * HALF],
                        lhsT=outt_sb[:, q, i * P:(i + 1) * P],
                        rhs=wp_sb[:, q, hlf * HALF:(hlf + 1) * HALF],
                        start=(q == 0), stop=(q == NP - 1),
                    )
            y_sb = y_pool.tile([P, D], F32, tag="y")
            nc.vector.tensor_add(y_sb, y_ps, bp_sb)
            nc.sync.dma_start(d_y[i * P:(i + 1) * P, :], y_sb)


_CACHE: dict = {}


def _get_program() -> bass.Bass:
    if "nc" not in _CACHE:
        _CACHE["nc"] = build_program()
    return _CACHE["nc"]


def _prep_inputs(sin, Wx, bx, Wp, bp):
    """Host-side sharding + layout prep. Returns per-core input maps."""
    bf16 = ml_dtypes.bfloat16
    wx_flat = np.ascontiguousarray(
        np.transpose(np.asarray(Wx, np.float32), (1, 0, 2)).reshape(D, H * F)
    ).astype(bf16)
    wp_b = np.ascontiguousarray(np.asarray(Wp, np.float32)).astype(bf16)
    bx32 = np.asarray(bx, np.float32)
    # bxp[p, q] = bx[2q + p//64, p%64]
    bxp = np.ascontiguousarray(bx32.reshape(NP, P).T)
    bxf = np.ascontiguousarray(bx32.reshape(1, H * F))
    bp32 = np.ascontiguousarray(np.asarray(bp, np.float32).reshape(1, D))
    # sel8[i][k][p] = 1 iff k == (p//64)*8 + i  (broadcasts rcpq rows i and
    # 8+i of a pair's [16,128] 1/rs tile to partitions 0-63 / 64-127)
    sel8 = np.zeros((2 * NT, NT, P), np.float32)
    for i in range(NT):
        sel8[i, i, :F] = 1.0
        sel8[NT + i, i, F:] = 1.0
    ident = np.eye(P, dtype=np.float32)

    sin32 = np.asarray(sin, np.float32)
    in_maps = []
    for b in range(B):
        sint = np.ascontiguousarray(sin32[b].T).astype(bf16)
        in_maps.append({
            "sint": sint, "wx": wx_flat, "wp": wp_b, "bxp": bxp, "bxf": bxf,
            "bp": bp32, "sel8": sel8, "ident": ident,
        })
    return in_maps


def kernel(sin, mask, Wx, bx, Wp, bp, _run_kwargs=None):
    nc = _get_program()
    in_maps = _prep_inputs(sin, Wx, bx, Wp, bp)
    res = run_bass_kernel_spmd(nc, in_maps, core_ids=list(range(B)),
                               **(_run_kwargs or {}))
    out = np.stack([np.asarray(res.results[b]["y"], np.float32) for b in range(B)])
    if _run_kwargs:
        _CACHE["last_results"] = res
    return out


# revision 35
# speedup vs baseline: 1.2006x; 1.1170x over previous
"""Trainium2 Bass kernel for nn_MultiHeadAttention_91027536871977.

Cosine-similarity multi-head self-attention:
  x      = einsum("bsd,hdf->bhsf", sin, Wx) + bx          [B,H,S,F]
  scores = (x @ x^T) / (|x| |x|^T)                        [B,H,S,S]
  p      = softmax(scores, -1)
  out    = concat_heads(p @ x) @ Wp + bp                  [B,S,D]

Sharding: pure data-parallel over batch (B=8 -> 8 cores, one batch each,
all 16 heads + the output projection local to the core; no collectives).

Per-core algorithm (S=1024, D=1024, H=16, F=64, P=128):
  - host pre-transposes sin[b] -> sinT [D,S] and casts weights to bf16
  - XT[f2,s] per head-pair q (2 heads stacked on 128 partitions) via matmul
  - n^2 per head via (XT*XT) summed over the 64 feature partitions with a
    0/1-mask matmul; inv_n = sqrt(1/n^2); XTn = XT * inv_n (cols scaled)
  - Gram G = XTn^T XTn per head (K=64, two heads run concurrently on
    disjoint PE row-groups) gives fully normalized scores (symmetric)
  - E = exp(G) on ScalarE, PSUM->SBUF bf16, with accum_out giving row sums
    rs for free.  (The reference's `score==0 -> -inf` quirk fires on ~4 of
    134M elements in fp32 and is numerically negligible; ignored.)
  - out^T = X^T E / rs using E's symmetry (stored [s,t] tiles reinterpreted
    as [t,s]), X = sin @ Wx computed directly in [t, hf] layout.
    1/rs is laid out via a PE transpose of the accumulated rs matrix and
    broadcast across partitions with K=1 ones-matmuls.
  - Y = out^T.T @ Wp + bp via matmul, bias added during PSUM->SBUF copy.
"""

import numpy as np
import ml_dtypes

import concourse.bass as bass
import concourse.bacc as bacc
import concourse.mybir as mybir
import concourse.tile as tile
from concourse.bass_utils import run_bass_kernel_spmd

B, S, D, H, F = 8, 1024, 1024, 16, 64
P = 128
NP = H // 2  # head pairs
KO = D // P  # k subtiles
NT = S // P  # s tiles
BF16 = mybir.dt.bfloat16
F32 = mybir.dt.float32


def build_program() -> bass.Bass:
    nc = bacc.Bacc("TRN2", target_bir_lowering=False, debug=False)

    # Per-core inputs (already sharded/prepped on host).
    d_sint = nc.dram_tensor("sint", [D, S], BF16, kind="ExternalInput")
    d_wx = nc.dram_tensor("wx", [D, H * F], BF16, kind="ExternalInput")
    d_wp = nc.dram_tensor("wp", [H * F, D], BF16, kind="ExternalInput")
    d_bxp = nc.dram_tensor("bxp", [P, NP], F32, kind="ExternalInput")  # pair bias
    d_bxf = nc.dram_tensor("bxf", [1, H * F], F32, kind="ExternalInput")
    d_bp = nc.dram_tensor("bp", [1, D], F32, kind="ExternalInput")
    d_sel8 = nc.dram_tensor("sel8", [2 * NT, NT, P], BF16, kind="ExternalInput")
    d_ident = nc.dram_tensor("ident", [P, P], BF16, kind="ExternalInput")
    d_y = nc.dram_tensor("y", [S, D], F32, kind="ExternalOutput")

    with tile.TileContext(nc) as tc:
        _body(tc, d_sint, d_wx, d_wp, d_bxp, d_bxf, d_bp,
              d_sel8, d_ident, d_y)
    nc.compile()
    return nc


def _bcast_rows(dram_ap, parts=P):
    """DMA access pattern replicating a [1, N] DRAM row across `parts` partitions."""
    return bass.AP(
        tensor=dram_ap.tensor,
        offset=dram_ap.offset,
        ap=[[0, parts]] + list(dram_ap.ap[1:]),
    )


def _body(tc, d_sint, d_wx, d_wp, d_bxp, d_bxf, d_bp,
          d_sel8, d_ident, d_y):
    nc = tc.nc
    from contextlib import ExitStack

    with ExitStack() as ctx:
        singles = ctx.enter_context(tc.tile_pool(name="singles", bufs=1))
        sq_pool = ctx.enter_context(tc.tile_pool(name="sq", bufs=2))
        e_pool = ctx.enter_context(tc.tile_pool(name="epool", bufs=2))
        b_pool = ctx.enter_context(tc.tile_pool(name="bpool", bufs=2))
        y_pool = ctx.enter_context(tc.tile_pool(name="ypool", bufs=2))

        ps_big = ctx.enter_context(tc.tile_pool(name="ps_big", bufs=2, space="PSUM"))
        ps_out = ctx.enter_context(tc.tile_pool(name="ps_out", bufs=1, space="PSUM"))
        ps_small = ctx.enter_context(tc.tile_pool(name="ps_small", bufs=2, space="PSUM"))

        # ---- load everything to SBUF ----
        sint_sb = singles.tile([P, KO, S], BF16)
        nc.sync.dma_start(sint_sb, d_sint.rearrange("(ko p) s -> p ko s", p=P))
        wx_sb = singles.tile([P, KO, H * F], BF16)
        nc.sync.dma_start(wx_sb, d_wx.rearrange("(ko p) n -> p ko n", p=P))
        wp_sb = singles.tile([P, KO, D], BF16)
        nc.sync.dma_start(wp_sb, d_wp.rearrange("(ko p) n -> p ko n", p=P))
        bxp_sb = singles.tile([P, NP], F32)
        nc.sync.dma_start(bxp_sb, d_bxp[:, :])
        bxf_sb = singles.tile([P, H * F], F32)
        nc.gpsimd.dma_start(bxf_sb, _bcast_rows(d_bxf[:, :]))
        bp_sb = singles.tile([P, D], F32)
        nc.gpsimd.dma_start(bp_sb, _bcast_rows(d_bp[:, :]))
        sel8_sb = singles.tile([2 * NT, NT, P], BF16)
        nc.sync.dma_start(sel8_sb, d_sel8[:, :, :])
        ident_sb = singles.tile([P, P], BF16)
        nc.sync.dma_start(ident_sb, d_ident[:, :])

        # persistent intermediates
        xt_sb = singles.tile([P, NP, S], BF16)     # [f2, pair, t]  x^T per pair
        xtn_sb = singles.tile([P, NP, S], BF16)    # normalized x^T
        x_sb = singles.tile([P, NT, H * F], BF16)  # [t_p, t_tile, hf]  x values
        outt_sb = singles.tile([P, NP, S], BF16)   # attention out^T [f2, pair, s]
        rs_sb = singles.tile([P, P], F32)          # rs[s_p, col h*8+i]
        n2s_sb = singles.tile([P, P], F32)         # |x|^2 [s_p, col h*8+i]
        nrcp_sb = singles.tile([P, P], F32)        # 1/|x|^2 (fp32 scratch)
        invs_sb = singles.tile([P, P], BF16)       # 1/|x| [s_p, col h*8+i]

        HALF = S // 2

        # ---- X = sin @ Wx + bx in [t, hf] layout, + per-head |x|^2 ----
        for i in range(NT):
            x_ps = ps_big.tile([P, H * F], F32, tag="big")
            for hlf in range(2):
                for ko in range(KO):
                    nc.tensor.matmul(
                        x_ps[:, hlf * HALF:(hlf + 1) * HALF],
                        lhsT=sint_sb[:, ko, i * P:(i + 1) * P],
                        rhs=wx_sb[:, ko, hlf * HALF:(hlf + 1) * HALF],
                        start=(ko == 0), stop=(ko == KO - 1),
                    )
            nc.vector.tensor_add(x_sb[:, i, :], x_ps, bxf_sb[:, :])
            xsq = sq_pool.tile([P, H * F], F32, tag="xsq")
            nc.vector.tensor_mul(xsq, x_sb[:, i, :], x_sb[:, i, :])
            # reduce over F per head -> [s_p, 16]; scatter to cols h*8+i
            nc.vector.reduce_sum(
                n2s_sb.rearrange("p (hh ii) -> p hh ii", ii=NT)[:, :, i],
                xsq.rearrange("p (hh f) -> p hh f", f=F),
                axis=mybir.AxisListType.X,
            )
        # 1/|x| for all heads/tiles at once (full-partition ops are fast)
        nc.vector.reciprocal(nrcp_sb, n2s_sb)
        nc.scalar.sqrt(invs_sb, nrcp_sb)

        # ---- per pair: XT, normalized XT, attention, out^T ----
        e_tiles = {}
        for q in range(NP):
            # XT = X^T via PE transposes of the pair's 128 feature columns
            for j in range(NT):
                xtt_ps = ps_small.tile([P, P], BF16, tag="small")
                nc.tensor.transpose(
                    xtt_ps, x_sb[:, j, q * P:(q + 1) * P], ident_sb)
                nc.vector.tensor_copy(xt_sb[:, q, j * P:(j + 1) * P], xtt_ps)

            # 1/|x| free-layout rows for this pair: transpose [128,16] slice
            invq_ps = ps_small.tile([2 * NT, P], BF16, tag="small")
            nc.tensor.transpose(
                invq_ps, invs_sb[:, q * 2 * NT:(q + 1) * 2 * NT], ident_sb)
            invq_sb = b_pool.tile([2 * NT, P], BF16, tag="rcpq")
            nc.vector.tensor_copy(invq_sb, invq_ps)
            # broadcast across the pair's partitions, tile by tile, and scale
            nrm_sb = sq_pool.tile([P, NT, P], BF16, tag="nrm")
            for i in range(NT):
                nrm_ps = ps_small.tile([P, P], F32, tag="small")
                nc.tensor.matmul(
                    nrm_ps, lhsT=sel8_sb[:, i, :], rhs=invq_sb,
                    start=True, stop=True,
                )
                nc.vector.tensor_copy(nrm_sb[:, i, :], nrm_ps)
            nc.vector.tensor_mul(
                xtn_sb[:, q, :], xt_sb[:, q, :],
                nrm_sb.rearrange("p a b -> p (a b)"))

            # Gram + exp per head (row-packed K=64 matmuls run concurrently)
            for hh in range(2):
                h = 2 * q + hh
                frows = slice(hh * F, (hh + 1) * F)
                e_sb = e_pool.tile([P, NT, S], BF16, tag="e")
                e_tiles[hh] = e_sb
                for i in range(NT):
                    g_ps = ps_big.tile([P, S], F32, tag="big")
                    for hlf in range(2):
                        nc.tensor.matmul(
                            g_ps[:, hlf * HALF:(hlf + 1) * HALF],
                            lhsT=xtn_sb[frows, q, i * P:(i + 1) * P],
                            rhs=xtn_sb[frows, q, hlf * HALF:(hlf + 1) * HALF],
                            start=True, stop=True,
                        )
                    nc.scalar.activation(
                        e_sb[:, i, :], g_ps,
                        mybir.ActivationFunctionType.Exp,
                        accum_out=rs_sb[:, h * NT + i:h * NT + i + 1],
                    )

            # 1/rs rows for the pair (reciprocal on full partitions, then T)
            rcps_sb = b_pool.tile([P, 2 * NT], F32, tag="rcps")
            nc.vector.reciprocal(
                rcps_sb, rs_sb[:, q * 2 * NT:(q + 1) * 2 * NT])
            rcpsb_sb = b_pool.tile([P, 2 * NT], BF16, tag="rcpsb")
            nc.vector.tensor_copy(rcpsb_sb, rcps_sb)
            rst_ps = ps_small.tile([2 * NT, P], BF16, tag="small")
            nc.tensor.transpose(rst_ps, rcpsb_sb, ident_sb)
            rcpq_sb = b_pool.tile([2 * NT, P], BF16, tag="rcpq")
            nc.vector.tensor_copy(rcpq_sb, rst_ps)
            # broadcast tiles staged into one [128, S] buffer (single mul below)
            brc_sb = sq_pool.tile([P, NT, P], BF16, tag="nrm")
            for i in range(NT):
                brc_ps = ps_small.tile([P, P], F32, tag="small")
                nc.tensor.matmul(
                    brc_ps, lhsT=sel8_sb[:, i, :], rhs=rcpq_sb,
                    start=True, stop=True,
                )
                nc.vector.tensor_copy(brc_sb[:, i, :], brc_ps)

            # out^T accumulation over t tiles, both heads col-packed
            ot_ps = ps_out.tile([P, S], F32, tag="ot")
            for hlf in range(2):
                for j in range(NT):
                    for hh2 in range(2):
                        nc.tensor.matmul(
                            ot_ps[hh2 * F:(hh2 + 1) * F,
                                  hlf * HALF:(hlf + 1) * HALF],
                            lhsT=x_sb[:, j, (2 * q + hh2) * F:(2 * q + hh2 + 1) * F],
                            rhs=e_tiles[hh2][:, j, hlf * HALF:(hlf + 1) * HALF],
                            start=(j == 0), stop=(j == NT - 1),
                            tile_position=(0, hh2 * F),
                            skip_group_check=True,
                        )
            nc.vector.tensor_mul(
                outt_sb[:, q, :],
                brc_sb.rearrange("p a b -> p (a b)"),
                ot_ps,
            )

        # ---- output projection Y = out^T.T @ Wp + bp ----
        for i in range(NT):
            y_ps = ps_big.tile([P, D], F32, tag="big")
            for hlf in range(2):
                for q in range(NP):
                    nc.tensor.matmul(
                        y_ps[:, hlf * HALF:(hlf + 1) * HALF],
                        lhsT=outt_sb[:, q, i * P:(i + 1) * P],
                        rhs=wp_sb[:, q, hlf * HALF:(hlf + 1) * HALF],
                        start=(q == 0), stop=(q == NP - 1),
                    )
            y_sb = y_pool.tile([P, D], F32, tag="y")
            nc.vector.tensor_add(y_sb, y_ps, bp_sb)
            nc.sync.dma_start(d_y[i * P:(i + 1) * P, :], y_sb)


_CACHE: dict = {}


def _get_program() -> bass.Bass:
    if "nc" not in _CACHE:
        _CACHE["nc"] = build_program()
    return _CACHE["nc"]


def _prep_inputs(sin, Wx, bx, Wp, bp):
    """Host-side sharding + layout prep. Returns per-core input maps."""
    bf16 = ml_dtypes.bfloat16
    wx_flat = np.ascontiguousarray(
        np.transpose(np.asarray(Wx, np.float32), (1, 0, 2)).reshape(D, H * F)
    ).astype(bf16)
    wp_b = np.ascontiguousarray(np.asarray(Wp, np.float32)).astype(bf16)
    bx32 = np.asarray(bx, np.float32)
    # bxp[p, q] = bx[2q + p//64, p%64]
    bxp = np.ascontiguousarray(bx32.reshape(NP, P).T)
    bxf = np.ascontiguousarray(bx32.reshape(1, H * F))
    bp32 = np.ascontiguousarray(np.asarray(bp, np.float32).reshape(1, D))
    # sel8[i][k][p] = 1 iff k == (p//64)*8 + i  (broadcasts rcpq rows i and
    # 8+i of a pair's [16,128] 1/rs tile to partitions 0-63 / 64-127)
    sel8 = np.zeros((2 * NT, NT, P), np.float32)
    for i in range(NT):
        sel8[i, i, :F] = 1.0
        sel8[NT + i, i, F:] = 1.0
    sel8 = sel8.astype(bf16)
    ident = np.eye(P, dtype=np.float32).astype(bf16)

    sin32 = np.asarray(sin, np.float32)
    in_maps = []
    for b in range(B):
        sint = np.ascontiguousarray(sin32[b].T).astype(bf16)
        in_maps.append({
            "sint": sint, "wx": wx_flat, "wp": wp_b, "bxp": bxp, "bxf": bxf,
            "bp": bp32, "sel8": sel8, "ident": ident,
        })
    return in_maps


def kernel(sin, mask, Wx, bx, Wp, bp, _run_kwargs=None):
    nc = _get_program()
    in_maps = _prep_inputs(sin, Wx, bx, Wp, bp)
    res = run_bass_kernel_spmd(nc, in_maps, core_ids=list(range(B)),
                               **(_run_kwargs or {}))
    out = np.stack([np.asarray(res.results[b]["y"], np.float32) for b in range(B)])
    if _run_kwargs:
        _CACHE["last_results"] = res
    return out


# revision 38
# speedup vs baseline: 1.2795x; 1.0657x over previous
"""Trainium2 Bass kernel for nn_MultiHeadAttention_91027536871977.

Cosine-similarity multi-head self-attention:
  x      = einsum("bsd,hdf->bhsf", sin, Wx) + bx          [B,H,S,F]
  scores = (x @ x^T) / (|x| |x|^T)                        [B,H,S,S]
  p      = softmax(scores, -1)
  out    = concat_heads(p @ x) @ Wp + bp                  [B,S,D]

Sharding: pure data-parallel over batch (B=8 -> 8 cores, one batch each,
all 16 heads + the output projection local to the core; no collectives).

Per-core algorithm (S=1024, D=1024, H=16, F=64, P=128):
  - host pre-transposes sin[b] -> sinT [D,S] and casts weights to bf16
  - XT[f2,s] per head-pair q (2 heads stacked on 128 partitions) via matmul
  - n^2 per head via (XT*XT) summed over the 64 feature partitions with a
    0/1-mask matmul; inv_n = sqrt(1/n^2); XTn = XT * inv_n (cols scaled)
  - Gram G = XTn^T XTn per head (K=64, two heads run concurrently on
    disjoint PE row-groups) gives fully normalized scores (symmetric)
  - E = exp(G) on ScalarE, PSUM->SBUF bf16, with accum_out giving row sums
    rs for free.  (The reference's `score==0 -> -inf` quirk fires on ~4 of
    134M elements in fp32 and is numerically negligible; ignored.)
  - out^T = X^T E / rs using E's symmetry (stored [s,t] tiles reinterpreted
    as [t,s]), X = sin @ Wx computed directly in [t, hf] layout.
    1/rs is laid out via a PE transpose of the accumulated rs matrix and
    broadcast across partitions with K=1 ones-matmuls.
  - Y = out^T.T @ Wp + bp via matmul, bias added during PSUM->SBUF copy.
"""

import numpy as np
import ml_dtypes

import concourse.bass as bass
import concourse.bacc as bacc
import concourse.mybir as mybir
import concourse.tile as tile
from concourse.bass_utils import run_bass_kernel_spmd

B, S, D, H, F = 8, 1024, 1024, 16, 64
P = 128
NP = H // 2  # head pairs
KO = D // P  # k subtiles
NT = S // P  # s tiles
BF16 = mybir.dt.bfloat16
F32 = mybir.dt.float32


def build_program() -> bass.Bass:
    nc = bacc.Bacc("TRN2", target_bir_lowering=False, debug=False)

    # Per-core inputs (already sharded/prepped on host).
    d_sint = nc.dram_tensor("sint", [D, S], BF16, kind="ExternalInput")
    d_wx = nc.dram_tensor("wx", [D, H * F], BF16, kind="ExternalInput")
    d_wp = nc.dram_tensor("wp", [H * F, D], BF16, kind="ExternalInput")
    d_bxp = nc.dram_tensor("bxp", [P, NP], F32, kind="ExternalInput")  # pair bias
    d_bxf = nc.dram_tensor("bxf", [1, H * F], F32, kind="ExternalInput")
    d_bp = nc.dram_tensor("bp", [1, D], F32, kind="ExternalInput")
    d_sel8 = nc.dram_tensor("sel8", [2 * NT, NT, P], BF16, kind="ExternalInput")
    d_ident = nc.dram_tensor("ident", [P, P], BF16, kind="ExternalInput")
    d_y = nc.dram_tensor("y", [S, D], F32, kind="ExternalOutput")

    with tile.TileContext(nc) as tc:
        _body(tc, d_sint, d_wx, d_wp, d_bxp, d_bxf, d_bp,
              d_sel8, d_ident, d_y)
    nc.compile()
    return nc


def _bcast_rows(dram_ap, parts=P):
    """DMA access pattern replicating a [1, N] DRAM row across `parts` partitions."""
    return bass.AP(
        tensor=dram_ap.tensor,
        offset=dram_ap.offset,
        ap=[[0, parts]] + list(dram_ap.ap[1:]),
    )


def _body(tc, d_sint, d_wx, d_wp, d_bxp, d_bxf, d_bp,
          d_sel8, d_ident, d_y):
    nc = tc.nc
    from contextlib import ExitStack

    with ExitStack() as ctx:
        singles = ctx.enter_context(tc.tile_pool(name="singles", bufs=1))
        sq_pool = ctx.enter_context(tc.tile_pool(name="sq", bufs=2))
        e_pool = ctx.enter_context(tc.tile_pool(name="epool", bufs=2))
        b_pool = ctx.enter_context(tc.tile_pool(name="bpool", bufs=2))
        y_pool = ctx.enter_context(tc.tile_pool(name="ypool", bufs=2))

        ps_big = ctx.enter_context(tc.tile_pool(name="ps_big", bufs=3, space="PSUM"))
        ps_small = ctx.enter_context(tc.tile_pool(name="ps_small", bufs=2, space="PSUM"))

        # ---- load everything to SBUF ----
        sint_sb = singles.tile([P, KO, S], BF16)
        nc.sync.dma_start(sint_sb, d_sint.rearrange("(ko p) s -> p ko s", p=P))
        wx_sb = singles.tile([P, KO, H * F], BF16)
        nc.sync.dma_start(wx_sb, d_wx.rearrange("(ko p) n -> p ko n", p=P))
        wp_sb = singles.tile([P, KO, D], BF16)
        nc.sync.dma_start(wp_sb, d_wp.rearrange("(ko p) n -> p ko n", p=P))
        bxp_sb = singles.tile([P, NP], F32)
        nc.sync.dma_start(bxp_sb, d_bxp[:, :])
        bxf_sb = singles.tile([P, H * F], F32)
        nc.gpsimd.dma_start(bxf_sb, _bcast_rows(d_bxf[:, :]))
        bp_sb = singles.tile([P, D], F32)
        nc.gpsimd.dma_start(bp_sb, _bcast_rows(d_bp[:, :]))
        sel8_sb = singles.tile([2 * NT, NT, P], BF16)
        nc.sync.dma_start(sel8_sb, d_sel8[:, :, :])
        ident_sb = singles.tile([P, P], BF16)
        nc.sync.dma_start(ident_sb, d_ident[:, :])

        # persistent intermediates
        xt_sb = singles.tile([P, NP, S], BF16)     # [f2, pair, t]  x^T per pair
        xtn_sb = singles.tile([P, NP, S], BF16)    # normalized x^T
        x_sb = singles.tile([P, NT, H * F], BF16)  # [t_p, t_tile, hf]  x values
        outt_sb = singles.tile([P, NP, S], BF16)   # attention out^T [f2, pair, s]
        rs_sb = singles.tile([P, P], F32)          # rs[s_p, col h*8+i]
        n2s_sb = singles.tile([P, P], F32)         # |x|^2 [s_p, col h*8+i]
        nrcp_sb = singles.tile([P, P], F32)        # 1/|x|^2 (fp32 scratch)
        invs_sb = singles.tile([P, P], BF16)       # 1/|x| [s_p, col h*8+i]

        HALF = S // 2

        # ---- X = sin @ Wx + bx in [t, hf] layout, + per-head |x|^2 ----
        for i in range(NT):
            x_ps = ps_big.tile([P, H * F], F32, tag="big")
            for hlf in range(2):
                for ko in range(KO):
                    nc.tensor.matmul(
                        x_ps[:, hlf * HALF:(hlf + 1) * HALF],
                        lhsT=sint_sb[:, ko, i * P:(i + 1) * P],
                        rhs=wx_sb[:, ko, hlf * HALF:(hlf + 1) * HALF],
                        start=(ko == 0), stop=(ko == KO - 1),
                    )
            nc.vector.tensor_add(x_sb[:, i, :], x_ps, bxf_sb[:, :])
            xsq = sq_pool.tile([P, H * F], F32, tag="xsq")
            nc.vector.tensor_mul(xsq, x_sb[:, i, :], x_sb[:, i, :])
            # reduce over F per head -> [s_p, 16]; scatter to cols h*8+i
            nc.vector.reduce_sum(
                n2s_sb.rearrange("p (hh ii) -> p hh ii", ii=NT)[:, :, i],
                xsq.rearrange("p (hh f) -> p hh f", f=F),
                axis=mybir.AxisListType.X,
            )
        # 1/|x| for all heads/tiles at once (full-partition ops are fast)
        nc.vector.reciprocal(nrcp_sb, n2s_sb)
        nc.scalar.sqrt(invs_sb, nrcp_sb)

        # ---- per pair: XT, normalized XT, attention, out^T ----
        e_tiles = {}
        for q in range(NP):
            # XT = X^T via PE transposes of the pair's 128 feature columns
            for j in range(NT):
                xtt_ps = ps_small.tile([P, P], BF16, tag="small")
                nc.tensor.transpose(
                    xtt_ps, x_sb[:, j, q * P:(q + 1) * P], ident_sb)
                nc.vector.tensor_copy(xt_sb[:, q, j * P:(j + 1) * P], xtt_ps)

            # 1/|x| free-layout rows for this pair: transpose [128,16] slice
            invq_ps = ps_small.tile([2 * NT, P], BF16, tag="small")
            nc.tensor.transpose(
                invq_ps, invs_sb[:, q * 2 * NT:(q + 1) * 2 * NT], ident_sb)
            invq_sb = b_pool.tile([2 * NT, P], BF16, tag="rcpq")
            nc.vector.tensor_copy(invq_sb, invq_ps)
            # broadcast across the pair's partitions, tile by tile, and scale
            nrm_sb = sq_pool.tile([P, NT, P], BF16, tag="nrm")
            for i in range(NT):
                nrm_ps = ps_small.tile([P, P], F32, tag="small")
                nc.tensor.matmul(
                    nrm_ps, lhsT=sel8_sb[:, i, :], rhs=invq_sb,
                    start=True, stop=True,
                )
                nc.vector.tensor_copy(nrm_sb[:, i, :], nrm_ps)
            nc.vector.tensor_mul(
                xtn_sb[:, q, :], xt_sb[:, q, :],
                nrm_sb.rearrange("p a b -> p (a b)"))

            # Gram + exp, the two heads' K=64 matmuls interleaved so they run
            # concurrently on disjoint PE row-groups
            for hh in range(2):
                e_tiles[hh] = e_pool.tile([P, NT, S], BF16, tag="e",
                                          name=f"e_{q}_{hh}")
            for i in range(NT):
                g_tiles = {}
                for hh in range(2):
                    g_tiles[hh] = ps_big.tile([P, S], F32, tag="big",
                                              name=f"g_{q}_{hh}_{i}")
                for hlf in range(2):
                    for hh in range(2):
                        frows = slice(hh * F, (hh + 1) * F)
                        nc.tensor.matmul(
                            g_tiles[hh][:, hlf * HALF:(hlf + 1) * HALF],
                            lhsT=xtn_sb[frows, q, i * P:(i + 1) * P],
                            rhs=xtn_sb[frows, q, hlf * HALF:(hlf + 1) * HALF],
                            start=True, stop=True,
                        )
                for hh in range(2):
                    h = 2 * q + hh
                    nc.scalar.activation(
                        e_tiles[hh][:, i, :], g_tiles[hh],
                        mybir.ActivationFunctionType.Exp,
                        accum_out=rs_sb[:, h * NT + i:h * NT + i + 1],
                    )

            # 1/rs rows for the pair (reciprocal on full partitions, then T)
            rcps_sb = b_pool.tile([P, 2 * NT], F32, tag="rcps")
            nc.vector.reciprocal(
                rcps_sb, rs_sb[:, q * 2 * NT:(q + 1) * 2 * NT])
            rcpsb_sb = b_pool.tile([P, 2 * NT], BF16, tag="rcpsb")
            nc.vector.tensor_copy(rcpsb_sb, rcps_sb)
            rst_ps = ps_small.tile([2 * NT, P], BF16, tag="small")
            nc.tensor.transpose(rst_ps, rcpsb_sb, ident_sb)
            rcpq_sb = b_pool.tile([2 * NT, P], BF16, tag="rcpq")
            nc.vector.tensor_copy(rcpq_sb, rst_ps)
            # broadcast tiles staged into one [128, S] buffer (single mul below)
            brc_sb = sq_pool.tile([P, NT, P], BF16, tag="nrm")
            for i in range(NT):
                brc_ps = ps_small.tile([P, P], F32, tag="small")
                nc.tensor.matmul(
                    brc_ps, lhsT=sel8_sb[:, i, :], rhs=rcpq_sb,
                    start=True, stop=True,
                )
                nc.vector.tensor_copy(brc_sb[:, i, :], brc_ps)

            # out^T accumulation over t tiles, both heads col-packed
            ot_ps = ps_big.tile([P, S], F32, tag="big", name=f"ot_{q}")
            for hlf in range(2):
                for j in range(NT):
                    for hh2 in range(2):
                        nc.tensor.matmul(
                            ot_ps[hh2 * F:(hh2 + 1) * F,
                                  hlf * HALF:(hlf + 1) * HALF],
                            lhsT=x_sb[:, j, (2 * q + hh2) * F:(2 * q + hh2 + 1) * F],
                            rhs=e_tiles[hh2][:, j, hlf * HALF:(hlf + 1) * HALF],
                            start=(j == 0), stop=(j == NT - 1),
                            tile_position=(0, hh2 * F),
                            skip_group_check=True,
                        )
            nc.vector.tensor_mul(
                outt_sb[:, q, :],
                brc_sb.rearrange("p a b -> p (a b)"),
                ot_ps,
            )

        # ---- output projection Y = out^T.T @ Wp + bp ----
        for i in range(NT):
            y_ps = ps_big.tile([P, D], F32, tag="big")
            for hlf in range(2):
                for q in range(NP):
                    nc.tensor.matmul(
                        y_ps[:, hlf * HALF:(hlf + 1) * HALF],
                        lhsT=outt_sb[:, q, i * P:(i + 1) * P],
                        rhs=wp_sb[:, q, hlf * HALF:(hlf + 1) * HALF],
                        start=(q == 0), stop=(q == NP - 1),
                    )
            y_sb = y_pool.tile([P, D], F32, tag="y")
            nc.vector.tensor_add(y_sb, y_ps, bp_sb)
            nc.sync.dma_start(d_y[i * P:(i + 1) * P, :], y_sb)


_CACHE: dict = {}


def _get_program() -> bass.Bass:
    if "nc" not in _CACHE:
        _CACHE["nc"] = build_program()
    return _CACHE["nc"]


def _prep_inputs(sin, Wx, bx, Wp, bp):
    """Host-side sharding + layout prep. Returns per-core input maps."""
    bf16 = ml_dtypes.bfloat16
    wx_flat = np.ascontiguousarray(
        np.transpose(np.asarray(Wx, np.float32), (1, 0, 2)).reshape(D, H * F)
    ).astype(bf16)
    wp_b = np.ascontiguousarray(np.asarray(Wp, np.float32)).astype(bf16)
    bx32 = np.asarray(bx, np.float32)
    # bxp[p, q] = bx[2q + p//64, p%64]
    bxp = np.ascontiguousarray(bx32.reshape(NP, P).T)
    bxf = np.ascontiguousarray(bx32.reshape(1, H * F))
    bp32 = np.ascontiguousarray(np.asarray(bp, np.float32).reshape(1, D))
    # sel8[i][k][p] = 1 iff k == (p//64)*8 + i  (broadcasts rcpq rows i and
    # 8+i of a pair's [16,128] 1/rs tile to partitions 0-63 / 64-127)
    sel8 = np.zeros((2 * NT, NT, P), np.float32)
    for i in range(NT):
        sel8[i, i, :F] = 1.0
        sel8[NT + i, i, F:] = 1.0
    sel8 = sel8.astype(bf16)
    ident = np.eye(P, dtype=np.float32).astype(bf16)

    sin32 = np.asarray(sin, np.float32)
    in_maps = []
    for b in range(B):
        sint = np.ascontiguousarray(sin32[b].T).astype(bf16)
        in_maps.append({
            "sint": sint, "wx": wx_flat, "wp": wp_b, "bxp": bxp, "bxf": bxf,
            "bp": bp32, "sel8": sel8, "ident": ident,
        })
    return in_maps


def kernel(sin, mask, Wx, bx, Wp, bp, _run_kwargs=None):
    nc = _get_program()
    in_maps = _prep_inputs(sin, Wx, bx, Wp, bp)
    res = run_bass_kernel_spmd(nc, in_maps, core_ids=list(range(B)),
                               **(_run_kwargs or {}))
    out = np.stack([np.asarray(res.results[b]["y"], np.float32) for b in range(B)])
    if _run_kwargs:
        _CACHE["last_results"] = res
    return out


# revision 39
# speedup vs baseline: 1.3039x; 1.0191x over previous
"""Trainium2 Bass kernel for nn_MultiHeadAttention_91027536871977.

Cosine-similarity multi-head self-attention:
  x      = einsum("bsd,hdf->bhsf", sin, Wx) + bx          [B,H,S,F]
  scores = (x @ x^T) / (|x| |x|^T)                        [B,H,S,S]
  p      = softmax(scores, -1)
  out    = concat_heads(p @ x) @ Wp + bp                  [B,S,D]

Sharding: pure data-parallel over batch (B=8 -> 8 cores, one batch each,
all 16 heads + the output projection local to the core; no collectives).

Per-core algorithm (S=1024, D=1024, H=16, F=64, P=128):
  - host pre-transposes sin[b] -> sinT [D,S] and casts weights to bf16
  - XT[f2,s] per head-pair q (2 heads stacked on 128 partitions) via matmul
  - n^2 per head via (XT*XT) summed over the 64 feature partitions with a
    0/1-mask matmul; inv_n = sqrt(1/n^2); XTn = XT * inv_n (cols scaled)
  - Gram G = XTn^T XTn per head (K=64, two heads run concurrently on
    disjoint PE row-groups) gives fully normalized scores (symmetric)
  - E = exp(G) on ScalarE, PSUM->SBUF bf16, with accum_out giving row sums
    rs for free.  (The reference's `score==0 -> -inf` quirk fires on ~4 of
    134M elements in fp32 and is numerically negligible; ignored.)
  - out^T = X^T E / rs using E's symmetry (stored [s,t] tiles reinterpreted
    as [t,s]), X = sin @ Wx computed directly in [t, hf] layout.
    1/rs is laid out via a PE transpose of the accumulated rs matrix and
    broadcast across partitions with K=1 ones-matmuls.
  - Y = out^T.T @ Wp + bp via matmul, bias added during PSUM->SBUF copy.
"""

import numpy as np
import ml_dtypes

import concourse.bass as bass
import concourse.bacc as bacc
import concourse.mybir as mybir
import concourse.tile as tile
from concourse.bass_utils import run_bass_kernel_spmd

B, S, D, H, F = 8, 1024, 1024, 16, 64
P = 128
NP = H // 2  # head pairs
KO = D // P  # k subtiles
NT = S // P  # s tiles
BF16 = mybir.dt.bfloat16
F32 = mybir.dt.float32


def build_program() -> bass.Bass:
    nc = bacc.Bacc("TRN2", target_bir_lowering=False, debug=False)

    # Per-core inputs (already sharded/prepped on host).
    d_sint = nc.dram_tensor("sint", [D, S], BF16, kind="ExternalInput")
    d_wx = nc.dram_tensor("wx", [D, H * F], BF16, kind="ExternalInput")
    d_wp = nc.dram_tensor("wp", [H * F, D], BF16, kind="ExternalInput")
    d_bxp = nc.dram_tensor("bxp", [P, NP], F32, kind="ExternalInput")  # pair bias
    d_bxf = nc.dram_tensor("bxf", [1, H * F], F32, kind="ExternalInput")
    d_bp = nc.dram_tensor("bp", [1, D], F32, kind="ExternalInput")
    d_sel8 = nc.dram_tensor("sel8", [2 * NT, NT, P], BF16, kind="ExternalInput")
    d_ident = nc.dram_tensor("ident", [P, P], BF16, kind="ExternalInput")
    d_y = nc.dram_tensor("y", [S, D], F32, kind="ExternalOutput")

    with tile.TileContext(nc) as tc:
        _body(tc, d_sint, d_wx, d_wp, d_bxp, d_bxf, d_bp,
              d_sel8, d_ident, d_y)
    nc.compile()
    return nc


def _bcast_rows(dram_ap, parts=P):
    """DMA access pattern replicating a [1, N] DRAM row across `parts` partitions."""
    return bass.AP(
        tensor=dram_ap.tensor,
        offset=dram_ap.offset,
        ap=[[0, parts]] + list(dram_ap.ap[1:]),
    )


def _body(tc, d_sint, d_wx, d_wp, d_bxp, d_bxf, d_bp,
          d_sel8, d_ident, d_y):
    nc = tc.nc
    from contextlib import ExitStack

    with ExitStack() as ctx:
        singles = ctx.enter_context(tc.tile_pool(name="singles", bufs=1))
        sq_pool = ctx.enter_context(tc.tile_pool(name="sq", bufs=2))
        e_pool = ctx.enter_context(tc.tile_pool(name="epool", bufs=4))
        b_pool = ctx.enter_context(tc.tile_pool(name="bpool", bufs=2))
        y_pool = ctx.enter_context(tc.tile_pool(name="ypool", bufs=2))
        bc_pool = ctx.enter_context(tc.tile_pool(name="bcpool", bufs=1))

        ps_big = ctx.enter_context(tc.tile_pool(name="ps_big", bufs=2, space="PSUM"))
        ps_out = ctx.enter_context(tc.tile_pool(name="ps_out", bufs=1, space="PSUM"))
        ps_small = ctx.enter_context(tc.tile_pool(name="ps_small", bufs=2, space="PSUM"))

        # ---- load everything to SBUF ----
        sint_sb = singles.tile([P, KO, S], BF16)
        nc.sync.dma_start(sint_sb, d_sint.rearrange("(ko p) s -> p ko s", p=P))
        wx_sb = singles.tile([P, KO, H * F], BF16)
        nc.sync.dma_start(wx_sb, d_wx.rearrange("(ko p) n -> p ko n", p=P))
        wp_sb = singles.tile([P, KO, D], BF16)
        nc.sync.dma_start(wp_sb, d_wp.rearrange("(ko p) n -> p ko n", p=P))
        bxf_sb = bc_pool.tile([P, H * F], F32, tag="bc", name="bxf_sb")
        nc.gpsimd.dma_start(bxf_sb, _bcast_rows(d_bxf[:, :]))
        sel8_sb = singles.tile([2 * NT, NT, P], BF16)
        nc.sync.dma_start(sel8_sb, d_sel8[:, :, :])
        ident_sb = singles.tile([P, P], BF16)
        nc.sync.dma_start(ident_sb, d_ident[:, :])

        # persistent intermediates
        xtn_sb = singles.tile([P, NP, S], BF16)    # normalized x^T [f2, pair, t]
        x_sb = singles.tile([P, NT, H * F], BF16)  # [t_p, t_tile, hf]  x values
        outt_sb = singles.tile([P, NP, S], BF16)   # attention out^T [f2, pair, s]
        rs_sb = singles.tile([P, P], F32)          # rs[s_p, col h*8+i]
        n2s_sb = singles.tile([P, P], F32)         # |x|^2 [s_p, col h*8+i]
        nrcp_sb = singles.tile([P, P], F32)        # 1/|x|^2 (fp32 scratch)
        invs_sb = singles.tile([P, P], BF16)       # 1/|x| [s_p, col h*8+i]

        HALF = S // 2

        # ---- X = sin @ Wx + bx in [t, hf] layout, + per-head |x|^2 ----
        for i in range(NT):
            x_ps = ps_big.tile([P, H * F], F32, tag="big", name=f"x_{i}")
            for hlf in range(2):
                for ko in range(KO):
                    nc.tensor.matmul(
                        x_ps[:, hlf * HALF:(hlf + 1) * HALF],
                        lhsT=sint_sb[:, ko, i * P:(i + 1) * P],
                        rhs=wx_sb[:, ko, hlf * HALF:(hlf + 1) * HALF],
                        start=(ko == 0), stop=(ko == KO - 1),
                    )
            nc.vector.tensor_add(x_sb[:, i, :], x_ps, bxf_sb[:, :])
            xsq = sq_pool.tile([P, H * F], BF16, tag="xsq", name=f"xsq_{i}")
            nc.vector.tensor_mul(xsq, x_sb[:, i, :], x_sb[:, i, :])
            # reduce over F per head -> [s_p, 16]; scatter to cols h*8+i
            nc.vector.reduce_sum(
                n2s_sb.rearrange("p (hh ii) -> p hh ii", ii=NT)[:, :, i],
                xsq.rearrange("p (hh f) -> p hh f", f=F),
                axis=mybir.AxisListType.X,
            )
        # 1/|x| for all heads/tiles at once (full-partition ops are fast)
        nc.vector.reciprocal(nrcp_sb, n2s_sb)
        nc.scalar.sqrt(invs_sb, nrcp_sb)

        e_store = {}

        def prep(q):
            """Normalized XT for pair q: 1/|x| broadcast + fused transpose-scale."""
            invq_ps = ps_small.tile([2 * NT, P], BF16, tag="small",
                                    name=f"invq_{q}")
            nc.tensor.transpose(
                invq_ps, invs_sb[:, q * 2 * NT:(q + 1) * 2 * NT], ident_sb)
            invq_sb = b_pool.tile([2 * NT, P], BF16, tag="rcpq",
                                  name=f"invqs_{q}")
            nc.vector.tensor_copy(invq_sb, invq_ps)
            nrm_sb = sq_pool.tile([P, NT, P], BF16, tag="nrm", name=f"nrm_{q}")
            for i in range(NT):
                nrm_ps = ps_small.tile([P, P], F32, tag="small",
                                       name=f"nrmp_{q}_{i}")
                nc.tensor.matmul(
                    nrm_ps, lhsT=sel8_sb[:, i, :], rhs=invq_sb,
                    start=True, stop=True,
                )
                nc.vector.tensor_copy(nrm_sb[:, i, :], nrm_ps)
            for j in range(NT):
                xtt_ps = ps_small.tile([P, P], BF16, tag="small",
                                       name=f"xtt_{q}_{j}")
                nc.tensor.transpose(
                    xtt_ps, x_sb[:, j, q * P:(q + 1) * P], ident_sb)
                nc.vector.tensor_mul(
                    xtn_sb[:, q, j * P:(j + 1) * P], xtt_ps, nrm_sb[:, j, :])

        def gram(q):
            """Gram + exp for both heads of pair q (A/B interleaved)."""
            e_store[q] = [
                e_pool.tile([P, NT, S], BF16, tag="e", name=f"e_{q}_{hh}")
                for hh in range(2)]
            for i in range(NT):
                g_tiles = [
                    ps_big.tile([P, S], F32, tag="big", name=f"g_{q}_{hh}_{i}")
                    for hh in range(2)]
                for hlf in range(2):
                    for hh in range(2):
                        frows = slice(hh * F, (hh + 1) * F)
                        nc.tensor.matmul(
                            g_tiles[hh][:, hlf * HALF:(hlf + 1) * HALF],
                            lhsT=xtn_sb[frows, q, i * P:(i + 1) * P],
                            rhs=xtn_sb[frows, q, hlf * HALF:(hlf + 1) * HALF],
                            start=True, stop=True,
                        )
                for hh in range(2):
                    h = 2 * q + hh
                    nc.scalar.activation(
                        e_store[q][hh][:, i, :], g_tiles[hh],
                        mybir.ActivationFunctionType.Exp,
                        accum_out=rs_sb[:, h * NT + i:h * NT + i + 1],
                    )

        def rs_chain(q):
            """1/rs broadcast tiles for pair q staged into brc_sb."""
            rcps_sb = b_pool.tile([P, 2 * NT], F32, tag="rcps",
                                  name=f"rcps_{q}")
            nc.vector.reciprocal(
                rcps_sb, rs_sb[:, q * 2 * NT:(q + 1) * 2 * NT])
            rcpsb_sb = b_pool.tile([P, 2 * NT], BF16, tag="rcpsb",
                                   name=f"rcpsb_{q}")
            nc.vector.tensor_copy(rcpsb_sb, rcps_sb)
            rst_ps = ps_small.tile([2 * NT, P], BF16, tag="small",
                                   name=f"rst_{q}")
            nc.tensor.transpose(rst_ps, rcpsb_sb, ident_sb)
            rcpq_sb = b_pool.tile([2 * NT, P], BF16, tag="rcpq",
                                  name=f"rcpq_{q}")
            nc.vector.tensor_copy(rcpq_sb, rst_ps)
            brc_sb = sq_pool.tile([P, NT, P], BF16, tag="nrm", name=f"brc_{q}")
            for i in range(NT):
                brc_ps = ps_small.tile([P, P], F32, tag="small",
                                       name=f"brcp_{q}_{i}")
                nc.tensor.matmul(
                    brc_ps, lhsT=sel8_sb[:, i, :], rhs=rcpq_sb,
                    start=True, stop=True,
                )
                nc.vector.tensor_copy(brc_sb[:, i, :], brc_ps)
            return brc_sb

        def ex(q):
            """out^T accumulation over t tiles, both heads col-packed."""
            ot_ps = ps_out.tile([P, S], F32, tag="ot", name=f"ot_{q}")
            for hlf in range(2):
                for j in range(NT):
                    for hh2 in range(2):
                        nc.tensor.matmul(
                            ot_ps[hh2 * F:(hh2 + 1) * F,
                                  hlf * HALF:(hlf + 1) * HALF],
                            lhsT=x_sb[:, j, (2 * q + hh2) * F:(2 * q + hh2 + 1) * F],
                            rhs=e_store[q][hh2][:, j, hlf * HALF:(hlf + 1) * HALF],
                            start=(j == 0), stop=(j == NT - 1),
                            tile_position=(0, hh2 * F),
                            skip_group_check=True,
                        )
            return ot_ps

        # ---- software-pipelined attention over pairs ----
        prep(0)
        gram(0)
        for q in range(NP):
            if q + 1 < NP:
                prep(q + 1)
            brc_sb = rs_chain(q)
            if q + 1 < NP:
                gram(q + 1)
            ot_ps = ex(q)
            nc.vector.tensor_mul(
                outt_sb[:, q, :],
                brc_sb.rearrange("p a b -> p (a b)"),
                ot_ps,
            )
            del e_store[q]

        # ---- output projection Y = out^T.T @ Wp + bp ----
        bp_sb = bc_pool.tile([P, D], F32, tag="bc", name="bp_sb")
        nc.gpsimd.dma_start(bp_sb, _bcast_rows(d_bp[:, :]))
        for i in range(NT):
            y_ps = ps_big.tile([P, D], F32, tag="big", name=f"y_{i}")
            for hlf in range(2):
                for q in range(NP):
                    nc.tensor.matmul(
                        y_ps[:, hlf * HALF:(hlf + 1) * HALF],
                        lhsT=outt_sb[:, q, i * P:(i + 1) * P],
                        rhs=wp_sb[:, q, hlf * HALF:(hlf + 1) * HALF],
                        start=(q == 0), stop=(q == NP - 1),
                    )
            y_sb = y_pool.tile([P, D], F32, tag="y", name=f"ys_{i}")
            nc.vector.tensor_add(y_sb, y_ps, bp_sb)
            nc.sync.dma_start(d_y[i * P:(i + 1) * P, :], y_sb)


_CACHE: dict = {}


def _get_program() -> bass.Bass:
    if "nc" not in _CACHE:
        _CACHE["nc"] = build_program()
    return _CACHE["nc"]


def _prep_inputs(sin, Wx, bx, Wp, bp):
    """Host-side sharding + layout prep. Returns per-core input maps."""
    bf16 = ml_dtypes.bfloat16
    wx_flat = np.ascontiguousarray(
        np.transpose(np.asarray(Wx, np.float32), (1, 0, 2)).reshape(D, H * F)
    ).astype(bf16)
    wp_b = np.ascontiguousarray(np.asarray(Wp, np.float32)).astype(bf16)
    bx32 = np.asarray(bx, np.float32)
    # bxp[p, q] = bx[2q + p//64, p%64]
    bxp = np.ascontiguousarray(bx32.reshape(NP, P).T)
    bxf = np.ascontiguousarray(bx32.reshape(1, H * F))
    bp32 = np.ascontiguousarray(np.asarray(bp, np.float32).reshape(1, D))
    # sel8[i][k][p] = 1 iff k == (p//64)*8 + i  (broadcasts rcpq rows i and
    # 8+i of a pair's [16,128] 1/rs tile to partitions 0-63 / 64-127)
    sel8 = np.zeros((2 * NT, NT, P), np.float32)
    for i in range(NT):
        sel8[i, i, :F] = 1.0
        sel8[NT + i, i, F:] = 1.0
    sel8 = sel8.astype(bf16)
    ident = np.eye(P, dtype=np.float32).astype(bf16)

    sin32 = np.asarray(sin, np.float32)
    in_maps = []
    for b in range(B):
        sint = np.ascontiguousarray(sin32[b].T).astype(bf16)
        in_maps.append({
            "sint": sint, "wx": wx_flat, "wp": wp_b, "bxp": bxp, "bxf": bxf,
            "bp": bp32, "sel8": sel8, "ident": ident,
        })
    return in_maps


def kernel(sin, mask, Wx, bx, Wp, bp, _run_kwargs=None):
    nc = _get_program()
    in_maps = _prep_inputs(sin, Wx, bx, Wp, bp)
    res = run_bass_kernel_spmd(nc, in_maps, core_ids=list(range(B)),
                               **(_run_kwargs or {}))
    out = np.stack([np.asarray(res.results[b]["y"], np.float32) for b in range(B)])
    if _run_kwargs:
        _CACHE["last_results"] = res
    return out


# revision 40
# speedup vs baseline: 1.4549x; 1.1158x over previous
"""Trainium2 Bass kernel for nn_MultiHeadAttention_91027536871977.

Cosine-similarity multi-head self-attention:
  x      = einsum("bsd,hdf->bhsf", sin, Wx) + bx          [B,H,S,F]
  scores = (x @ x^T) / (|x| |x|^T)                        [B,H,S,S]
  p      = softmax(scores, -1)
  out    = concat_heads(p @ x) @ Wp + bp                  [B,S,D]

Sharding: pure data-parallel over batch (B=8 -> 8 cores, one batch each,
all 16 heads + the output projection local to the core; no collectives).

Per-core algorithm (S=1024, D=1024, H=16, F=64, P=128):
  - host pre-transposes sin[b] -> sinT [D,S] and casts weights to bf16
  - XT[f2,s] per head-pair q (2 heads stacked on 128 partitions) via matmul
  - n^2 per head via (XT*XT) summed over the 64 feature partitions with a
    0/1-mask matmul; inv_n = sqrt(1/n^2); XTn = XT * inv_n (cols scaled)
  - Gram G = XTn^T XTn per head (K=64, two heads run concurrently on
    disjoint PE row-groups) gives fully normalized scores (symmetric)
  - E = exp(G) on ScalarE, PSUM->SBUF bf16, with accum_out giving row sums
    rs for free.  (The reference's `score==0 -> -inf` quirk fires on ~4 of
    134M elements in fp32 and is numerically negligible; ignored.)
  - out^T = X^T E / rs using E's symmetry (stored [s,t] tiles reinterpreted
    as [t,s]), X = sin @ Wx computed directly in [t, hf] layout.
    1/rs is laid out via a PE transpose of the accumulated rs matrix and
    broadcast across partitions with K=1 ones-matmuls.
  - Y = out^T.T @ Wp + bp via matmul, bias added during PSUM->SBUF copy.
"""

import numpy as np
import ml_dtypes

import concourse.bass as bass
import concourse.bacc as bacc
import concourse.mybir as mybir
import concourse.tile as tile
from concourse.bass_utils import run_bass_kernel_spmd

B, S, D, H, F = 8, 1024, 1024, 16, 64
P = 128
NP = H // 2  # head pairs
KO = D // P  # k subtiles
NT = S // P  # s tiles
BF16 = mybir.dt.bfloat16
F32 = mybir.dt.float32


def build_program() -> bass.Bass:
    nc = bacc.Bacc("TRN2", target_bir_lowering=False, debug=False)

    # Per-core inputs (already sharded/prepped on host).
    d_sint = nc.dram_tensor("sint", [D, S], BF16, kind="ExternalInput")
    d_wx = nc.dram_tensor("wx", [D, H * F], BF16, kind="ExternalInput")
    d_wp = nc.dram_tensor("wp", [H * F, D], BF16, kind="ExternalInput")
    d_bxp = nc.dram_tensor("bxp", [P, NP], F32, kind="ExternalInput")  # pair bias
    d_bxf = nc.dram_tensor("bxf", [1, H * F], F32, kind="ExternalInput")
    d_bp = nc.dram_tensor("bp", [1, D], F32, kind="ExternalInput")
    d_sel8 = nc.dram_tensor("sel8", [2 * NT, NT, P], BF16, kind="ExternalInput")
    d_ident = nc.dram_tensor("ident", [P, P], BF16, kind="ExternalInput")
    d_y = nc.dram_tensor("y", [S, D], F32, kind="ExternalOutput")

    with tile.TileContext(nc) as tc:
        _body(tc, d_sint, d_wx, d_wp, d_bxp, d_bxf, d_bp,
              d_sel8, d_ident, d_y)
    nc.compile()
    return nc


def _bcast_rows(dram_ap, parts=P):
    """DMA access pattern replicating a [1, N] DRAM row across `parts` partitions."""
    return bass.AP(
        tensor=dram_ap.tensor,
        offset=dram_ap.offset,
        ap=[[0, parts]] + list(dram_ap.ap[1:]),
    )


def _body(tc, d_sint, d_wx, d_wp, d_bxp, d_bxf, d_bp,
          d_sel8, d_ident, d_y):
    nc = tc.nc
    from contextlib import ExitStack

    with ExitStack() as ctx:
        singles = ctx.enter_context(tc.tile_pool(name="singles", bufs=1))
        sq_pool = ctx.enter_context(tc.tile_pool(name="sq", bufs=2))
        e_pool = ctx.enter_context(tc.tile_pool(name="epool", bufs=4))
        b_pool = ctx.enter_context(tc.tile_pool(name="bpool", bufs=2))
        y_pool = ctx.enter_context(tc.tile_pool(name="ypool", bufs=2))
        bc_pool = ctx.enter_context(tc.tile_pool(name="bcpool", bufs=1))

        ps_big = ctx.enter_context(tc.tile_pool(name="ps_big", bufs=3, space="PSUM"))
        ps_small = ctx.enter_context(tc.tile_pool(name="ps_small", bufs=2, space="PSUM"))

        # ---- load everything to SBUF ----
        sint_sb = singles.tile([P, KO, S], BF16)
        wx_sb = singles.tile([P, KO, H * F], BF16)
        sint_r = d_sint.rearrange("(ko p) s -> p ko s", p=P)
        wx_r = d_wx.rearrange("(ko p) n -> p ko n", p=P)
        for ko in range(KO):
            nc.sync.dma_start(wx_sb[:, ko, :], wx_r[:, ko, :])
            nc.sync.dma_start(sint_sb[:, ko, :], sint_r[:, ko, :])
        wp_sb = singles.tile([P, KO, D], BF16)
        nc.sync.dma_start(wp_sb, d_wp.rearrange("(ko p) n -> p ko n", p=P))
        bxf_sb = bc_pool.tile([P, H * F], F32, tag="bc", name="bxf_sb")
        nc.gpsimd.dma_start(bxf_sb, _bcast_rows(d_bxf[:, :]))
        sel8_sb = singles.tile([2 * NT, NT, P], BF16)
        nc.sync.dma_start(sel8_sb, d_sel8[:, :, :])
        ident_sb = singles.tile([P, P], BF16)
        nc.sync.dma_start(ident_sb, d_ident[:, :])

        # persistent intermediates
        xtn_sb = singles.tile([P, NP, S], BF16)    # normalized x^T [f2, pair, t]
        x_sb = singles.tile([P, NT, H * F], BF16)  # [t_p, t_tile, hf]  x values
        outt_sb = singles.tile([P, NP, S], BF16)   # attention out^T [f2, pair, s]
        rs_sb = singles.tile([P, P], F32)          # rs[s_p, col h*8+i]
        n2s_sb = singles.tile([P, P], F32)         # |x|^2 [s_p, col h*8+i]
        nrcp_sb = singles.tile([P, P], F32)        # 1/|x|^2 (fp32 scratch)
        invs_sb = singles.tile([P, P], BF16)       # 1/|x| [s_p, col h*8+i]

        HALF = S // 2

        # ---- X = sin @ Wx + bx in [t, hf] layout, + per-head |x|^2 ----
        for i in range(NT):
            x_ps = ps_big.tile([P, H * F], F32, tag="big", name=f"x_{i}")
            for hlf in range(2):
                for ko in range(KO):
                    nc.tensor.matmul(
                        x_ps[:, hlf * HALF:(hlf + 1) * HALF],
                        lhsT=sint_sb[:, ko, i * P:(i + 1) * P],
                        rhs=wx_sb[:, ko, hlf * HALF:(hlf + 1) * HALF],
                        start=(ko == 0), stop=(ko == KO - 1),
                    )
            nc.vector.tensor_add(x_sb[:, i, :], x_ps, bxf_sb[:, :])
            xsq = sq_pool.tile([P, H * F], BF16, tag="xsq", name=f"xsq_{i}")
            nc.vector.tensor_mul(xsq, x_sb[:, i, :], x_sb[:, i, :])
            # reduce over F per head -> [s_p, 16]; scatter to cols h*8+i
            nc.vector.reduce_sum(
                n2s_sb.rearrange("p (hh ii) -> p hh ii", ii=NT)[:, :, i],
                xsq.rearrange("p (hh f) -> p hh f", f=F),
                axis=mybir.AxisListType.X,
            )
        # 1/|x| for all heads/tiles at once (full-partition ops are fast)
        nc.vector.reciprocal(nrcp_sb, n2s_sb)
        nc.scalar.sqrt(invs_sb, nrcp_sb)

        e_store = {}

        def prep(q):
            """Normalized XT for pair q: 1/|x| broadcast + fused transpose-scale."""
            invq_ps = ps_small.tile([2 * NT, P], BF16, tag="small",
                                    name=f"invq_{q}")
            nc.tensor.transpose(
                invq_ps, invs_sb[:, q * 2 * NT:(q + 1) * 2 * NT], ident_sb)
            invq_sb = b_pool.tile([2 * NT, P], BF16, tag="rcpq",
                                  name=f"invqs_{q}")
            nc.vector.tensor_copy(invq_sb, invq_ps)
            nrm_sb = sq_pool.tile([P, NT, P], BF16, tag="nrm", name=f"nrm_{q}")
            for i in range(NT):
                nrm_ps = ps_small.tile([P, P], F32, tag="small",
                                       name=f"nrmp_{q}_{i}")
                nc.tensor.matmul(
                    nrm_ps, lhsT=sel8_sb[:, i, :], rhs=invq_sb,
                    start=True, stop=True,
                )
                nc.vector.tensor_copy(nrm_sb[:, i, :], nrm_ps)
            for j in range(NT):
                xtt_ps = ps_small.tile([P, P], BF16, tag="small",
                                       name=f"xtt_{q}_{j}")
                nc.tensor.transpose(
                    xtt_ps, x_sb[:, j, q * P:(q + 1) * P], ident_sb)
                nc.vector.tensor_mul(
                    xtn_sb[:, q, j * P:(j + 1) * P], xtt_ps, nrm_sb[:, j, :])

        def gram(q):
            """Gram + exp for both heads of pair q (A/B interleaved)."""
            e_store[q] = [
                e_pool.tile([P, NT, S], BF16, tag="e", name=f"e_{q}_{hh}")
                for hh in range(2)]
            for i in range(NT):
                g_tiles = [
                    ps_big.tile([P, S], F32, tag="big", name=f"g_{q}_{hh}_{i}")
                    for hh in range(2)]
                for hlf in range(2):
                    for hh in range(2):
                        frows = slice(hh * F, (hh + 1) * F)
                        nc.tensor.matmul(
                            g_tiles[hh][:, hlf * HALF:(hlf + 1) * HALF],
                            lhsT=xtn_sb[frows, q, i * P:(i + 1) * P],
                            rhs=xtn_sb[frows, q, hlf * HALF:(hlf + 1) * HALF],
                            start=True, stop=True,
                        )
                for hh in range(2):
                    h = 2 * q + hh
                    nc.scalar.activation(
                        e_store[q][hh][:, i, :], g_tiles[hh],
                        mybir.ActivationFunctionType.Exp,
                        accum_out=rs_sb[:, h * NT + i:h * NT + i + 1],
                    )

        def rs_chain(q):
            """1/rs broadcast tiles for pair q staged into brc_sb."""
            rcps_sb = b_pool.tile([P, 2 * NT], F32, tag="rcps",
                                  name=f"rcps_{q}")
            nc.vector.reciprocal(
                rcps_sb, rs_sb[:, q * 2 * NT:(q + 1) * 2 * NT])
            rcpsb_sb = b_pool.tile([P, 2 * NT], BF16, tag="rcpsb",
                                   name=f"rcpsb_{q}")
            nc.vector.tensor_copy(rcpsb_sb, rcps_sb)
            rst_ps = ps_small.tile([2 * NT, P], BF16, tag="small",
                                   name=f"rst_{q}")
            nc.tensor.transpose(rst_ps, rcpsb_sb, ident_sb)
            rcpq_sb = b_pool.tile([2 * NT, P], BF16, tag="rcpq",
                                  name=f"rcpq_{q}")
            nc.vector.tensor_copy(rcpq_sb, rst_ps)
            brc_sb = sq_pool.tile([P, NT, P], BF16, tag="nrm", name=f"brc_{q}")
            for i in range(NT):
                brc_ps = ps_small.tile([P, P], F32, tag="small",
                                       name=f"brcp_{q}_{i}")
                nc.tensor.matmul(
                    brc_ps, lhsT=sel8_sb[:, i, :], rhs=rcpq_sb,
                    start=True, stop=True,
                )
                nc.vector.tensor_copy(brc_sb[:, i, :], brc_ps)
            return brc_sb

        def ex(q, brc_sb):
            """out^T accumulation over t tiles, both heads col-packed.
            Half-width PSUM tiles; rescale fused into the copy-out."""
            for hlf in range(2):
                ot_ps = ps_small.tile([P, HALF], F32, tag="small",
                                      name=f"ot_{q}_{hlf}")
                for j in range(NT):
                    for hh2 in range(2):
                        nc.tensor.matmul(
                            ot_ps[hh2 * F:(hh2 + 1) * F, :],
                            lhsT=x_sb[:, j, (2 * q + hh2) * F:(2 * q + hh2 + 1) * F],
                            rhs=e_store[q][hh2][:, j, hlf * HALF:(hlf + 1) * HALF],
                            start=(j == 0), stop=(j == NT - 1),
                            tile_position=(0, hh2 * F),
                            skip_group_check=True,
                        )
                nc.vector.tensor_mul(
                    outt_sb[:, q, hlf * HALF:(hlf + 1) * HALF],
                    brc_sb.rearrange("p a b -> p (a b)")[:, hlf * HALF:(hlf + 1) * HALF],
                    ot_ps,
                )

        # ---- software-pipelined attention over pairs ----
        for q in range(NP):
            prep(q)
        gram(0)
        for q in range(NP):
            brc_sb = rs_chain(q)
            if q + 1 < NP:
                gram(q + 1)
            ex(q, brc_sb)
            del e_store[q]

        # ---- output projection Y = out^T.T @ Wp + bp ----
        bp_sb = bc_pool.tile([P, D], F32, tag="bc", name="bp_sb")
        nc.gpsimd.dma_start(bp_sb, _bcast_rows(d_bp[:, :]))
        for i in range(NT):
            y_ps = ps_big.tile([P, D], F32, tag="big", name=f"y_{i}")
            for hlf in range(2):
                for q in range(NP):
                    nc.tensor.matmul(
                        y_ps[:, hlf * HALF:(hlf + 1) * HALF],
                        lhsT=outt_sb[:, q, i * P:(i + 1) * P],
                        rhs=wp_sb[:, q, hlf * HALF:(hlf + 1) * HALF],
                        start=(q == 0), stop=(q == NP - 1),
                    )
            y_sb = y_pool.tile([P, D], F32, tag="y", name=f"ys_{i}")
            nc.vector.tensor_add(y_sb, y_ps, bp_sb)
            nc.sync.dma_start(d_y[i * P:(i + 1) * P, :], y_sb)


_CACHE: dict = {}


def _get_program() -> bass.Bass:
    if "nc" not in _CACHE:
        _CACHE["nc"] = build_program()
    return _CACHE["nc"]


def _prep_inputs(sin, Wx, bx, Wp, bp):
    """Host-side sharding + layout prep. Returns per-core input maps."""
    bf16 = ml_dtypes.bfloat16
    wx_flat = np.ascontiguousarray(
        np.transpose(np.asarray(Wx, np.float32), (1, 0, 2)).reshape(D, H * F)
    ).astype(bf16)
    wp_b = np.ascontiguousarray(np.asarray(Wp, np.float32)).astype(bf16)
    bx32 = np.asarray(bx, np.float32)
    # bxp[p, q] = bx[2q + p//64, p%64]
    bxp = np.ascontiguousarray(bx32.reshape(NP, P).T)
    bxf = np.ascontiguousarray(bx32.reshape(1, H * F))
    bp32 = np.ascontiguousarray(np.asarray(bp, np.float32).reshape(1, D))
    # sel8[i][k][p] = 1 iff k == (p//64)*8 + i  (broadcasts rcpq rows i and
    # 8+i of a pair's [16,128] 1/rs tile to partitions 0-63 / 64-127)
    sel8 = np.zeros((2 * NT, NT, P), np.float32)
    for i in range(NT):
        sel8[i, i, :F] = 1.0
        sel8[NT + i, i, F:] = 1.0
    sel8 = sel8.astype(bf16)
    ident = np.eye(P, dtype=np.float32).astype(bf16)

    sin32 = np.asarray(sin, np.float32)
    in_maps = []
    for b in range(B):
        sint = np.ascontiguousarray(sin32[b].T).astype(bf16)
        in_maps.append({
            "sint": sint, "wx": wx_flat, "wp": wp_b, "bxp": bxp, "bxf": bxf,
            "bp": bp32, "sel8": sel8, "ident": ident,
        })
    return in_maps


def kernel(sin, mask, Wx, bx, Wp, bp, _run_kwargs=None):
    nc = _get_program()
    in_maps = _prep_inputs(sin, Wx, bx, Wp, bp)
    res = run_bass_kernel_spmd(nc, in_maps, core_ids=list(range(B)),
                               **(_run_kwargs or {}))
    out = np.stack([np.asarray(res.results[b]["y"], np.float32) for b in range(B)])
    if _run_kwargs:
        _CACHE["last_results"] = res
    return out


# revision 41
# speedup vs baseline: 1.5734x; 1.0814x over previous
"""Trainium2 Bass kernel for nn_MultiHeadAttention_91027536871977.

Cosine-similarity multi-head self-attention:
  x      = einsum("bsd,hdf->bhsf", sin, Wx) + bx          [B,H,S,F]
  scores = (x @ x^T) / (|x| |x|^T)                        [B,H,S,S]
  p      = softmax(scores, -1)
  out    = concat_heads(p @ x) @ Wp + bp                  [B,S,D]

Sharding: pure data-parallel over batch (B=8 -> 8 cores, one batch each,
all 16 heads + the output projection local to the core; no collectives).

Per-core algorithm (S=1024, D=1024, H=16, F=64, P=128):
  - host pre-transposes sin[b] -> sinT [D,S] and casts weights to bf16
  - XT[f2,s] per head-pair q (2 heads stacked on 128 partitions) via matmul
  - n^2 per head via (XT*XT) summed over the 64 feature partitions with a
    0/1-mask matmul; inv_n = sqrt(1/n^2); XTn = XT * inv_n (cols scaled)
  - Gram G = XTn^T XTn per head (K=64, two heads run concurrently on
    disjoint PE row-groups) gives fully normalized scores (symmetric)
  - E = exp(G) on ScalarE, PSUM->SBUF bf16, with accum_out giving row sums
    rs for free.  (The reference's `score==0 -> -inf` quirk fires on ~4 of
    134M elements in fp32 and is numerically negligible; ignored.)
  - out^T = X^T E / rs using E's symmetry (stored [s,t] tiles reinterpreted
    as [t,s]), X = sin @ Wx computed directly in [t, hf] layout.
    1/rs is laid out via a PE transpose of the accumulated rs matrix and
    broadcast across partitions with K=1 ones-matmuls.
  - Y = out^T.T @ Wp + bp via matmul, bias added during PSUM->SBUF copy.
"""

import numpy as np
import ml_dtypes

import concourse.bass as bass
import concourse.bacc as bacc
import concourse.mybir as mybir
import concourse.tile as tile
from concourse.bass_utils import run_bass_kernel_spmd

B, S, D, H, F = 8, 1024, 1024, 16, 64
P = 128
NP = H // 2  # head pairs
KO = D // P  # k subtiles
NT = S // P  # s tiles
BF16 = mybir.dt.bfloat16
F32 = mybir.dt.float32


def build_program() -> bass.Bass:
    nc = bacc.Bacc("TRN2", target_bir_lowering=False, debug=False)

    # Per-core inputs (already sharded/prepped on host).
    d_sint = nc.dram_tensor("sint", [D, S], BF16, kind="ExternalInput")
    d_wx = nc.dram_tensor("wx", [D, H * F], BF16, kind="ExternalInput")
    d_wp = nc.dram_tensor("wp", [H * F, D], BF16, kind="ExternalInput")
    d_bxp = nc.dram_tensor("bxp", [P, NP], F32, kind="ExternalInput")  # pair bias
    d_bxf = nc.dram_tensor("bxf", [1, H * F], F32, kind="ExternalInput")
    d_bp = nc.dram_tensor("bp", [1, D], F32, kind="ExternalInput")
    d_sel8 = nc.dram_tensor("sel8", [2 * NT, NT, P], BF16, kind="ExternalInput")
    d_ident = nc.dram_tensor("ident", [P, P], BF16, kind="ExternalInput")
    d_y = nc.dram_tensor("y", [S, D], F32, kind="ExternalOutput")

    with tile.TileContext(nc) as tc:
        _body(tc, d_sint, d_wx, d_wp, d_bxp, d_bxf, d_bp,
              d_sel8, d_ident, d_y)
    nc.compile()
    return nc


def _bcast_rows(dram_ap, parts=P):
    """DMA access pattern replicating a [1, N] DRAM row across `parts` partitions."""
    return bass.AP(
        tensor=dram_ap.tensor,
        offset=dram_ap.offset,
        ap=[[0, parts]] + list(dram_ap.ap[1:]),
    )


def _body(tc, d_sint, d_wx, d_wp, d_bxp, d_bxf, d_bp,
          d_sel8, d_ident, d_y):
    nc = tc.nc
    from contextlib import ExitStack

    with ExitStack() as ctx:
        singles = ctx.enter_context(tc.tile_pool(name="singles", bufs=1))
        sq_pool = ctx.enter_context(tc.tile_pool(name="sq", bufs=2))
        e_pool = ctx.enter_context(tc.tile_pool(name="epool", bufs=4))
        b_pool = ctx.enter_context(tc.tile_pool(name="bpool", bufs=2))
        y_pool = ctx.enter_context(tc.tile_pool(name="ypool", bufs=2))
        bc_pool = ctx.enter_context(tc.tile_pool(name="bcpool", bufs=1))

        ps_big = ctx.enter_context(tc.tile_pool(name="ps_big", bufs=3, space="PSUM"))
        ps_small = ctx.enter_context(tc.tile_pool(name="ps_small", bufs=2, space="PSUM"))

        # ---- load everything to SBUF ----
        sint_sb = singles.tile([P, KO, S], BF16)
        wx_sb = singles.tile([P, KO, H * F], BF16)
        sint_r = d_sint.rearrange("(ko p) s -> p ko s", p=P)
        wx_r = d_wx.rearrange("(ko p) n -> p ko n", p=P)
        for ko in range(KO):
            nc.sync.dma_start(wx_sb[:, ko, :], wx_r[:, ko, :])
            nc.sync.dma_start(sint_sb[:, ko, :], sint_r[:, ko, :])
        wp_sb = singles.tile([P, KO, D], BF16)
        nc.sync.dma_start(wp_sb, d_wp.rearrange("(ko p) n -> p ko n", p=P))
        bxf_sb = bc_pool.tile([P, H * F], F32, tag="bc", name="bxf_sb")
        nc.gpsimd.dma_start(bxf_sb, _bcast_rows(d_bxf[:, :]))
        sel8_sb = singles.tile([2 * NT, NT, P], BF16)
        nc.sync.dma_start(sel8_sb, d_sel8[:, :, :])
        ident_sb = singles.tile([P, P], BF16)
        nc.sync.dma_start(ident_sb, d_ident[:, :])

        # persistent intermediates
        xtn_sb = singles.tile([P, NP, S], BF16)    # normalized x^T [f2, pair, t]
        x_sb = singles.tile([P, NT, H * F], BF16)  # [t_p, t_tile, hf]  x values
        outt_sb = singles.tile([P, NP, S], BF16)   # attention out^T [f2, pair, s]
        rs_sb = singles.tile([P, P], F32)          # rs[s_p, col h*8+i]
        n2s_sb = singles.tile([P, P], F32)         # |x|^2 [s_p, col h*8+i]
        nrcp_sb = singles.tile([P, P], F32)        # 1/|x|^2 (fp32 scratch)
        invs_sb = singles.tile([P, P], BF16)       # 1/|x| [s_p, col h*8+i]

        HALF = S // 2

        # ---- X = sin @ Wx + bx in [t, hf] layout, + per-head |x|^2 ----
        for i in range(NT):
            x_ps = ps_big.tile([P, H * F], F32, tag="big", name=f"x_{i}")
            for hlf in range(2):
                for ko in range(KO):
                    nc.tensor.matmul(
                        x_ps[:, hlf * HALF:(hlf + 1) * HALF],
                        lhsT=sint_sb[:, ko, i * P:(i + 1) * P],
                        rhs=wx_sb[:, ko, hlf * HALF:(hlf + 1) * HALF],
                        start=(ko == 0), stop=(ko == KO - 1),
                    )
            nc.vector.tensor_add(x_sb[:, i, :], x_ps, bxf_sb[:, :])
            xsq = sq_pool.tile([P, H * F], BF16, tag="xsq", name=f"xsq_{i}")
            nc.vector.tensor_mul(xsq, x_sb[:, i, :], x_sb[:, i, :])
            # reduce over F per head -> [s_p, 16]; scatter to cols h*8+i
            nc.vector.reduce_sum(
                n2s_sb.rearrange("p (hh ii) -> p hh ii", ii=NT)[:, :, i],
                xsq.rearrange("p (hh f) -> p hh f", f=F),
                axis=mybir.AxisListType.X,
            )
        # 1/|x| for all heads/tiles at once (full-partition ops are fast)
        nc.vector.reciprocal(nrcp_sb, n2s_sb)
        nc.scalar.sqrt(invs_sb, nrcp_sb)

        e_store = {}

        def prep(q):
            """Normalized XT for pair q: 1/|x| broadcast + fused transpose-scale."""
            invq_ps = ps_small.tile([2 * NT, P], BF16, tag="small",
                                    name=f"invq_{q}")
            nc.tensor.transpose(
                invq_ps, invs_sb[:, q * 2 * NT:(q + 1) * 2 * NT], ident_sb)
            invq_sb = b_pool.tile([2 * NT, P], BF16, tag="rcpq",
                                  name=f"invqs_{q}")
            nc.vector.tensor_copy(invq_sb, invq_ps)
            nrm_sb = sq_pool.tile([P, NT, P], BF16, tag="nrm", name=f"nrm_{q}")
            for i in range(NT):
                nrm_ps = ps_small.tile([P, P], F32, tag="small",
                                       name=f"nrmp_{q}_{i}")
                nc.tensor.matmul(
                    nrm_ps, lhsT=sel8_sb[:, i, :], rhs=invq_sb,
                    start=True, stop=True,
                )
                nc.vector.tensor_copy(nrm_sb[:, i, :], nrm_ps)
            for j in range(NT):
                xtt_ps = ps_small.tile([P, P], BF16, tag="small",
                                       name=f"xtt_{q}_{j}")
                nc.tensor.transpose(
                    xtt_ps, x_sb[:, j, q * P:(q + 1) * P], ident_sb)
                nc.vector.tensor_mul(
                    xtn_sb[:, q, j * P:(j + 1) * P], xtt_ps, nrm_sb[:, j, :])

        def gram_tile(q, i):
            """Gram + exp for both heads of pair q at s-tile i."""
            g_tiles = [
                ps_big.tile([P, S], F32, tag="big", name=f"g_{q}_{hh}_{i}")
                for hh in range(2)]
            for hlf in range(2):
                for hh in range(2):
                    frows = slice(hh * F, (hh + 1) * F)
                    nc.tensor.matmul(
                        g_tiles[hh][:, hlf * HALF:(hlf + 1) * HALF],
                        lhsT=xtn_sb[frows, q, i * P:(i + 1) * P],
                        rhs=xtn_sb[frows, q, hlf * HALF:(hlf + 1) * HALF],
                        start=True, stop=True,
                    )
            for hh in range(2):
                h = 2 * q + hh
                nc.scalar.activation(
                    e_store[q][hh][:, i, :], g_tiles[hh],
                    mybir.ActivationFunctionType.Exp,
                    accum_out=rs_sb[:, h * NT + i:h * NT + i + 1],
                )

        def gram(q):
            e_store[q] = [
                e_pool.tile([P, NT, S], BF16, tag="e", name=f"e_{q}_{hh}")
                for hh in range(2)]
            for i in range(NT):
                gram_tile(q, i)

        def rs_chain(q):
            """1/rs broadcast tiles for pair q staged into brc_sb."""
            rcps_sb = b_pool.tile([P, 2 * NT], F32, tag="rcps",
                                  name=f"rcps_{q}")
            nc.vector.reciprocal(
                rcps_sb, rs_sb[:, q * 2 * NT:(q + 1) * 2 * NT])
            rcpsb_sb = b_pool.tile([P, 2 * NT], BF16, tag="rcpsb",
                                   name=f"rcpsb_{q}")
            nc.vector.tensor_copy(rcpsb_sb, rcps_sb)
            rst_ps = ps_small.tile([2 * NT, P], BF16, tag="small",
                                   name=f"rst_{q}")
            nc.tensor.transpose(rst_ps, rcpsb_sb, ident_sb)
            rcpq_sb = b_pool.tile([2 * NT, P], BF16, tag="rcpq",
                                  name=f"rcpq_{q}")
            nc.vector.tensor_copy(rcpq_sb, rst_ps)
            brc_sb = sq_pool.tile([P, NT, P], BF16, tag="nrm", name=f"brc_{q}")
            for i in range(NT):
                brc_ps = ps_small.tile([P, P], F32, tag="small",
                                       name=f"brcp_{q}_{i}")
                nc.tensor.matmul(
                    brc_ps, lhsT=sel8_sb[:, i, :], rhs=rcpq_sb,
                    start=True, stop=True,
                )
                nc.vector.tensor_copy(brc_sb[:, i, :], brc_ps)
            return brc_sb

        def ex_half(q, hlf, brc_sb, nxt):
            """Half of out^T accumulation for pair q, with pair nxt's gram
            tiles interleaved into the PE stream to keep ACT fed."""
            ot_ps = ps_small.tile([P, HALF], F32, tag="small",
                                  name=f"ot_{q}_{hlf}")
            for j in range(NT):
                if nxt is not None and j % 2 == 0:
                    gram_tile(nxt, hlf * 4 + j // 2)
                for hh2 in range(2):
                    nc.tensor.matmul(
                        ot_ps[hh2 * F:(hh2 + 1) * F, :],
                        lhsT=x_sb[:, j, (2 * q + hh2) * F:(2 * q + hh2 + 1) * F],
                        rhs=e_store[q][hh2][:, j, hlf * HALF:(hlf + 1) * HALF],
                        start=(j == 0), stop=(j == NT - 1),
                        tile_position=(0, hh2 * F),
                        skip_group_check=True,
                    )
            nc.vector.tensor_mul(
                outt_sb[:, q, hlf * HALF:(hlf + 1) * HALF],
                brc_sb.rearrange("p a b -> p (a b)")[:, hlf * HALF:(hlf + 1) * HALF],
                ot_ps,
            )

        # ---- software-pipelined attention over pairs ----
        for q in range(NP):
            prep(q)
        gram(0)
        for q in range(NP):
            brc_sb = rs_chain(q)
            nxt = q + 1 if q + 1 < NP else None
            if nxt is not None:
                e_store[nxt] = [
                    e_pool.tile([P, NT, S], BF16, tag="e", name=f"e_{nxt}_{hh}")
                    for hh in range(2)]
            for hlf in range(2):
                ex_half(q, hlf, brc_sb, nxt)
            del e_store[q]

        # ---- output projection Y = out^T.T @ Wp + bp ----
        bp_sb = bc_pool.tile([P, D], F32, tag="bc", name="bp_sb")
        nc.gpsimd.dma_start(bp_sb, _bcast_rows(d_bp[:, :]))
        for i in range(NT):
            y_ps = ps_big.tile([P, D], F32, tag="big", name=f"y_{i}")
            for hlf in range(2):
                for q in range(NP):
                    nc.tensor.matmul(
                        y_ps[:, hlf * HALF:(hlf + 1) * HALF],
                        lhsT=outt_sb[:, q, i * P:(i + 1) * P],
                        rhs=wp_sb[:, q, hlf * HALF:(hlf + 1) * HALF],
                        start=(q == 0), stop=(q == NP - 1),
                    )
            y_sb = y_pool.tile([P, D], F32, tag="y", name=f"ys_{i}")
            nc.vector.tensor_add(y_sb, y_ps, bp_sb)
            nc.sync.dma_start(d_y[i * P:(i + 1) * P, :], y_sb)


_CACHE: dict = {}


def _get_program() -> bass.Bass:
    if "nc" not in _CACHE:
        _CACHE["nc"] = build_program()
    return _CACHE["nc"]


def _prep_inputs(sin, Wx, bx, Wp, bp):
    """Host-side sharding + layout prep. Returns per-core input maps."""
    bf16 = ml_dtypes.bfloat16
    wx_flat = np.ascontiguousarray(
        np.transpose(np.asarray(Wx, np.float32), (1, 0, 2)).reshape(D, H * F)
    ).astype(bf16)
    wp_b = np.ascontiguousarray(np.asarray(Wp, np.float32)).astype(bf16)
    bx32 = np.asarray(bx, np.float32)
    # bxp[p, q] = bx[2q + p//64, p%64]
    bxp = np.ascontiguousarray(bx32.reshape(NP, P).T)
    bxf = np.ascontiguousarray(bx32.reshape(1, H * F))
    bp32 = np.ascontiguousarray(np.asarray(bp, np.float32).reshape(1, D))
    # sel8[i][k][p] = 1 iff k == (p//64)*8 + i  (broadcasts rcpq rows i and
    # 8+i of a pair's [16,128] 1/rs tile to partitions 0-63 / 64-127)
    sel8 = np.zeros((2 * NT, NT, P), np.float32)
    for i in range(NT):
        sel8[i, i, :F] = 1.0
        sel8[NT + i, i, F:] = 1.0
    sel8 = sel8.astype(bf16)
    ident = np.eye(P, dtype=np.float32).astype(bf16)

    sin32 = np.asarray(sin, np.float32)
    in_maps = []
    for b in range(B):
        sint = np.ascontiguousarray(sin32[b].T).astype(bf16)
        in_maps.append({
            "sint": sint, "wx": wx_flat, "wp": wp_b, "bxp": bxp, "bxf": bxf,
            "bp": bp32, "sel8": sel8, "ident": ident,
        })
    return in_maps


def kernel(sin, mask, Wx, bx, Wp, bp, _run_kwargs=None):
    nc = _get_program()
    in_maps = _prep_inputs(sin, Wx, bx, Wp, bp)
    res = run_bass_kernel_spmd(nc, in_maps, core_ids=list(range(B)),
                               **(_run_kwargs or {}))
    out = np.stack([np.asarray(res.results[b]["y"], np.float32) for b in range(B)])
    if _run_kwargs:
        _CACHE["last_results"] = res
    return out
